# revision 10
# baseline (speedup 1.0000x reference)
"""GateAttentionUnit Trainium2 kernel.

Full inputs in, full output out. Data-parallel over batch: 16 batches
split 2-per-core across 8 NeuronCores; each core runs an identical NEFF
(SPMD) on its own x slice with replicated weights.

Device math per core (2 sequential batches of [512, 1024]):
  LN (ln affine folded into uv weights on host) -> xn, PE-transposed to
  xnT [d,n] -> uv projection via fp32r matmuls (base head / v natural /
  u transposed orientations) -> silu -> rope'd q,k (host sin/cos
  tables) -> qkT -> kernelT = relu(qkT + biasT)^2 (toeplitz bias table
  from host) -> kvT -> g = u * kv -> out proj + bias + shortcut.
"""

import sys

if "/opt/trn_rl_repo" not in sys.path:
    sys.path.insert(0, "/opt/trn_rl_repo")

import numpy as np

MAX_SEQ = 512
HIDDEN = 1024
E = 2048
S = 128
EPS = 1e-5
N_CORES = 8
B = 16
B_LOC = B // N_CORES  # 2 batches per core
P = 128
NLOC = B_LOC * MAX_SEQ  # 1024 rows per core

_CACHE = {}


class _Ctx:
    pass


def _phase0_ln(c, b):
    """LayerNorm batch b and transpose into c.xnT [d-part, n-free]."""
    import concourse.mybir as mybir

    nc = c.nc
    f32 = mybir.dt.float32
    f32r = mybir.dt.float32r
    Alu = mybir.AluOpType
    Act = mybir.ActivationFunctionType
    r0 = b * MAX_SEQ
    with c.tc.tile_pool(name="psA", bufs=4, space="PSUM") as psA:
        for nt in range(4):
            xt = c.x_pool.tile([P, HIDDEN], f32, tag="xt")
            nc.sync.dma_start(xt[:], c.x_ap[r0 + nt * P : r0 + (nt + 1) * P, :])
            s1 = c.stats.tile([P, 1], f32, tag="s1")
            nc.vector.tensor_reduce(s1[:], xt[:], mybir.AxisListType.X, Alu.add)
            sq = c.x_pool.tile([P, HIDDEN], f32, tag="sq")
            s2 = c.stats.tile([P, 1], f32, tag="s2")
            nc.scalar.activation(sq[:], xt[:], Act.Square, accum_out=s2[:])
            mu = c.stats.tile([P, 1], f32, tag="mu")
            nc.vector.tensor_scalar_mul(mu[:], s1[:], 1.0 / HIDDEN)
            m2 = c.stats.tile([P, 1], f32, tag="m2")
            nc.vector.tensor_scalar_mul(m2[:], s2[:], 1.0 / HIDDEN)
            # var = m2 - mu^2
            var = c.stats.tile([P, 1], f32, tag="var")
            mu2 = c.stats.tile([P, 1], f32, tag="mu2")
            nc.vector.tensor_mul(mu2[:], mu[:], mu[:])
            nc.vector.tensor_sub(var[:], m2[:], mu2[:])
            sd = c.stats.tile([P, 1], f32, tag="sd")
            nc.scalar.activation(sd[:], var[:], Act.Sqrt, bias=c.eps_sb[:])
            rstd = c.stats.tile([P, 1], f32, tag="rstd")
            nc.vector.reciprocal(rstd[:], sd[:])
            nmu = c.stats.tile([P, 1], f32, tag="nmu")
            nc.vector.scalar_tensor_tensor(
                nmu[:], mu[:], -1.0, rstd[:], Alu.mult, Alu.mult
            )  # (-mu) * rstd
            xn = c.x_pool.tile([P, HIDDEN], f32, tag="sq")
            nc.vector.tensor_scalar(xn[:], xt[:], rstd[:], nmu[:], Alu.mult, Alu.add)
            for dt_i in range(8):
                ps = psA.tile([P, P], f32, tag="tp")
                nc.tensor.transpose(ps[:], xn[:, dt_i * P : (dt_i + 1) * P], c.ident[:])
                nc.vector.tensor_copy(c.xnT[:, dt_i, nt * P : (nt + 1) * P], ps[:])


def _phase1_kernelT(c):
    """base head -> silu -> scale-offset -> rope -> qkT -> kernelT."""
    import concourse.mybir as mybir

    nc = c.nc
    f32 = mybir.dt.float32
    f32r = mybir.dt.float32r
    Alu = mybir.AluOpType
    Act = mybir.ActivationFunctionType
    with c.tc.tile_pool(name="psB", bufs=2, space="PSUM") as psB:
        wb = c.wbase_pool.tile([P, 8, S], f32r, tag="wb")
        nc.sync.dma_start(
            wb[:], c.uvw[:, 2 * E : 2 * E + S].rearrange("(do di) f -> di do f", di=P)
        )
        bps = psB.tile([P, MAX_SEQ], f32, tag="bps")
        for k in range(8):
            nc.tensor.matmul(
                bps[:], (wb[:, k, :]), (c.xnT[:, k, :]),
                start=(k == 0), stop=(k == 7),
            )
        base_sb = c.qk1_pool.tile([P, MAX_SEQ], f32, tag="base")
        nc.scalar.activation(base_sb[:], bps[:], Act.Silu, bias=c.bb_sb[:, 0:1])
        q_ro = c.qk1_pool.tile([P, MAX_SEQ], f32r, tag="q_ro")
        k_ro = c.qk1_pool.tile([P, MAX_SEQ], f32r, tag="k_ro")
        for gb_sb, ro in ((c.gbq_sb, q_ro), (c.gbk_sb, k_ro)):
            pre = c.qk_pool.tile([P, MAX_SEQ], f32, tag="pre")
            nc.vector.tensor_scalar(
                pre[:], base_sb[:], gb_sb[:, 0:1], gb_sb[:, 1:2], Alu.mult, Alu.add
            )
            sw = c.qk_pool.tile([P, MAX_SEQ], f32, tag="sw")
            nc.sync.dma_start(sw[0:64, :], pre[64:128, :])
            nc.sync.dma_start(sw[64:128, :], pre[0:64, :])
            tmp = c.qk_pool.tile([P, MAX_SEQ], f32, tag="rtmp")
            nc.vector.tensor_mul(tmp[:], sw[:], c.sin_sb[:])
            nc.vector.tensor_mul(ro[:], pre[:], c.cos_sb[:])
            nc.vector.tensor_add(ro[:], ro[:], tmp[:])
        kernelT = c.kern_pool.tile([P, 4, MAX_SEQ], f32r, tag="kernelT")
        for mt in range(4):
            qps = psB.tile([P, MAX_SEQ], f32, tag="qps")
            nc.tensor.matmul(
                qps[:], (k_ro[:, mt * P : (mt + 1) * P]), (q_ro[:]),
                start=True, stop=True,
            )
            t = c.qk_pool.tile([P, MAX_SEQ], f32, tag="kt_tmp")
            nc.vector.tensor_add(t[:], qps[:], c.bias_sb[:, mt, :])
            # relu(t)^2 == max(t,0)*t
            nc.vector.scalar_tensor_tensor(
                kernelT[:, mt, :], t[:], 0.0, t[:], Alu.max, Alu.mult
            )
        c.kernelT = kernelT


def _phase2_gated(c):
    """v chunks, u tiles, kvT, g = u * kv into c.gT."""
    import concourse.mybir as mybir

    nc = c.nc
    f32 = mybir.dt.float32
    f32r = mybir.dt.float32r
    Act = mybir.ActivationFunctionType
    with c.tc.tile_pool(name="psC", bufs=2, space="PSUM") as psC:
        for ec in range(4):
            wv = c.wv_pool.tile([P, 8, 512], f32r, tag="wv")
            nc.sync.dma_start(
                wv[:],
                c.uvw[:, E + ec * 512 : E + (ec + 1) * 512].rearrange(
                    "(do di) f -> di do f", di=P
                ),
            )
            vch = c.v_pool.tile([P, 4, 512], f32r, tag="vch")
            for nt in range(4):
                vps = psC.tile([P, 512], f32, tag="vps")
                for k in range(8):
                    nc.tensor.matmul(
                        vps[:], (c.xnT[:, k, nt * P : (nt + 1) * P]),
                        (wv[:, k, :]),
                        start=(k == 0), stop=False,
                    )
                nc.tensor.matmul(
                    vps[:], (c.ones_sb[:]),
                    (c.vb_sb[:, ec * 512 : (ec + 1) * 512]),
                    start=False, stop=True,
                )
                nc.scalar.activation(vch[:, nt, :], vps[:], Act.Silu)
            for et in range(4):
                ei = ec * 4 + et
                wu = c.wu_pool.tile([P, 8, P], f32r, tag="wu")
                nc.sync.dma_start(
                    wu[:],
                    c.uvw[:, ei * P : (ei + 1) * P].rearrange(
                        "(do di) f -> di do f", di=P
                    ),
                )
                ups = psC.tile([P, MAX_SEQ], f32, tag="ups")
                for k in range(8):
                    nc.tensor.matmul(
                        ups[:], (wu[:, k, :]), (c.xnT[:, k, :]),
                        start=(k == 0), stop=(k == 7),
                    )
                ut = c.u_pool.tile([P, MAX_SEQ], f32, tag="ut")
                nc.scalar.activation(
                    ut[:], ups[:], Act.Silu, bias=c.ub_sb[:, ei : ei + 1]
                )
                kps = psC.tile([P, MAX_SEQ], f32, tag="kps")
                for mt in range(4):
                    nc.tensor.matmul(
                        kps[:], (vch[:, mt, et * P : (et + 1) * P]),
                        (c.kernelT[:, mt, :]),
                        start=(mt == 0), stop=(mt == 3),
                    )
                nc.vector.tensor_mul(c.gT[:, ei, :], ut[:], kps[:])


def _phase3_out(c, b):
    """out = gT.T @ o_wT + o_b + shortcut, DMA to DRAM."""
    import concourse.mybir as mybir

    nc = c.nc
    f32 = mybir.dt.float32
    f32r = mybir.dt.float32r
    r0 = b * MAX_SEQ
    with c.tc.tile_pool(name="psD", bufs=8, space="PSUM") as psD:
        accs = [psD.tile([P, 512], f32, tag="acc", name=f"acc{i}") for i in range(8)]
        for et in range(16):
            wo = c.wo_pool.tile([P, 2, 512], f32r, tag="wo")
            nc.sync.dma_start(
                wo[:],
                c.owt[et * P : (et + 1) * P, :].rearrange("p (dc f) -> p dc f", dc=2),
            )
            for nt in range(4):
                for dc in range(2):
                    nc.tensor.matmul(
                        accs[nt * 2 + dc][:],
                        (c.gT[:, et, nt * P : (nt + 1) * P]),
                        (wo[:, dc, :]),
                        start=(et == 0), stop=False,
                    )
        for nt in range(4):
            for dc in range(2):
                nc.tensor.matmul(
                    accs[nt * 2 + dc][:], (c.ones_sb[:]),
                    (c.ob_sb[:, dc * 512 : (dc + 1) * 512]),
                    start=False, stop=True,
                )
                osb = c.out_pool.tile([P, 512], f32, tag="osb")
                xs = c.xs_pool.tile([P, 512], f32, tag="xs")
                nc.sync.dma_start(
                    xs[:],
                    c.x_ap[r0 + nt * P : r0 + (nt + 1) * P, dc * 512 : (dc + 1) * 512],
                )
                nc.vector.tensor_add(osb[:], accs[nt * 2 + dc][:], xs[:])
                nc.sync.dma_start(
                    c.out_ap[
                        r0 + nt * P : r0 + (nt + 1) * P, dc * 512 : (dc + 1) * 512
                    ],
                    osb[:],
                )


def _build():
    import concourse.mybir as mybir
    import concourse.tile as tile
    from concourse import bacc
    from concourse.masks import make_identity

    f32 = mybir.dt.float32
    f32r = mybir.dt.float32r

    nc = bacc.Bacc("TRN2", target_bir_lowering=False, debug=False,
                   num_devices=N_CORES)

    x_d = nc.dram_tensor("x", [NLOC, HIDDEN], f32, kind="ExternalInput")
    uvw_d = nc.dram_tensor("uv_wT", [HIDDEN, 2 * E + S], f32r, kind="ExternalInput")
    owt_d = nc.dram_tensor("o_wT", [E, HIDDEN], f32r, kind="ExternalInput")
    bias_d = nc.dram_tensor("biasT", [P, 4, MAX_SEQ], f32, kind="ExternalInput")
    cos_d = nc.dram_tensor("cos", [P, MAX_SEQ], f32, kind="ExternalInput")
    sin_d = nc.dram_tensor("sin", [P, MAX_SEQ], f32, kind="ExternalInput")
    ub_d = nc.dram_tensor("ub", [P, 16], f32, kind="ExternalInput")
    vb_d = nc.dram_tensor("vb", [1, E], f32r, kind="ExternalInput")
    bb_d = nc.dram_tensor("bb", [P, 1], f32, kind="ExternalInput")
    ob_d = nc.dram_tensor("ob", [1, HIDDEN], f32r, kind="ExternalInput")
    gbq_d = nc.dram_tensor("gbq", [P, 2], f32, kind="ExternalInput")
    gbk_d = nc.dram_tensor("gbk", [P, 2], f32, kind="ExternalInput")
    ones_d = nc.dram_tensor("ones", [1, P], f32r, kind="ExternalInput")
    out_d = nc.dram_tensor("out", [NLOC, HIDDEN], f32, kind="ExternalOutput")

    c = _Ctx()
    c.nc = nc
    c.x_ap = x_d.ap()
    c.out_ap = out_d.ap()
    c.uvw = uvw_d.ap()
    c.owt = owt_d.ap()

    with tile.TileContext(nc) as tc:
        c.tc = tc
        with (
            tc.tile_pool(name="const", bufs=1) as const,
            tc.tile_pool(name="xt", bufs=2) as x_pool,
            tc.tile_pool(name="stats", bufs=4) as stats,
            tc.tile_pool(name="xnT", bufs=1) as xnT_pool,
            tc.tile_pool(name="wbase", bufs=1) as wbase_pool,
            tc.tile_pool(name="qk1", bufs=1) as qk1_pool,
            tc.tile_pool(name="qk2", bufs=2) as qk_pool,
            tc.tile_pool(name="kern", bufs=1) as kern_pool,
            tc.tile_pool(name="wv", bufs=2) as wv_pool,
            tc.tile_pool(name="vch", bufs=2) as v_pool,
            tc.tile_pool(name="wu", bufs=2) as wu_pool,
            tc.tile_pool(name="ut", bufs=3) as u_pool,
            tc.tile_pool(name="gT", bufs=1) as gT_pool,
            tc.tile_pool(name="wo", bufs=2) as wo_pool,
            tc.tile_pool(name="outs", bufs=2) as out_pool,
            tc.tile_pool(name="xs", bufs=2) as xs_pool,
        ):
            c.x_pool = x_pool
            c.stats = stats
            c.wbase_pool = wbase_pool
            c.qk1_pool = qk1_pool
            c.qk_pool = qk_pool
            c.kern_pool = kern_pool
            c.wv_pool = wv_pool
            c.v_pool = v_pool
            c.wu_pool = wu_pool
            c.u_pool = u_pool
            c.wo_pool = wo_pool
            c.out_pool = out_pool
            c.xs_pool = xs_pool

            c.ident = const.tile([P, P], f32)
            make_identity(nc, c.ident[:])
            c.cos_sb = const.tile([P, MAX_SEQ], f32)
            nc.sync.dma_start(c.cos_sb[:], cos_d.ap())
            c.sin_sb = const.tile([P, MAX_SEQ], f32)
            nc.sync.dma_start(c.sin_sb[:], sin_d.ap())
            c.bias_sb = const.tile([P, 4, MAX_SEQ], f32)
            nc.sync.dma_start(c.bias_sb[:], bias_d.ap())
            c.ub_sb = const.tile([P, 16], f32)
            nc.sync.dma_start(c.ub_sb[:], ub_d.ap())
            c.vb_sb = const.tile([1, E], f32r)
            nc.sync.dma_start(c.vb_sb[:], vb_d.ap())
            c.bb_sb = const.tile([P, 1], f32)
            nc.sync.dma_start(c.bb_sb[:], bb_d.ap())
            c.ob_sb = const.tile([1, HIDDEN], f32r)
            nc.sync.dma_start(c.ob_sb[:], ob_d.ap())
            c.gbq_sb = const.tile([P, 2], f32)
            nc.sync.dma_start(c.gbq_sb[:], gbq_d.ap())
            c.gbk_sb = const.tile([P, 2], f32)
            nc.sync.dma_start(c.gbk_sb[:], gbk_d.ap())
            c.ones_sb = const.tile([1, P], f32r)
            nc.sync.dma_start(c.ones_sb[:], ones_d.ap())
            c.eps_sb = const.tile([P, 1], f32)
            nc.vector.memset(c.eps_sb[:], EPS)

            for b in range(B_LOC):
                c.xnT = xnT_pool.tile([P, 8, MAX_SEQ], f32r, tag="xnT", name="xnT")
                _phase0_ln(c, b)
                _phase1_kernelT(c)
                c.gT = gT_pool.tile([P, 16, MAX_SEQ], f32r, tag="gT", name="gT")
                _phase2_gated(c)
                _phase3_out(c, b)

    nc.compile()
    return nc


def _host_prep(x, ln_w, ln_b, uv_w, uv_b, gamma, beta, o_w, o_b, w_rel):
    f = np.float32
    x = np.ascontiguousarray(x, dtype=f)
    # fold LN affine into the uv projection (exact algebra)
    uv_wT = np.ascontiguousarray((uv_w * ln_w[None, :]).T, dtype=f)
    uv_b_eff = (
        uv_b.astype(np.float64) + uv_w.astype(np.float64) @ ln_b.astype(np.float64)
    ).astype(f)
    o_wT = np.ascontiguousarray(o_w.T, dtype=f)
    # toeplitz bias, transposed orientation: biasT[m, n] = bias[n, m] = w_rel[511 + m - n]
    idx = (MAX_SEQ - 1) + np.arange(MAX_SEQ)[:, None] - np.arange(MAX_SEQ)[None, :]
    biasT = w_rel[idx].astype(f)
    biasT_dev = np.ascontiguousarray(biasT.reshape(4, P, MAX_SEQ).transpose(1, 0, 2))
    # rope tables (match reference: fp32 sinusoid, accurate sin/cos)
    pos = np.arange(MAX_SEQ, dtype=f)
    half = S // 2
    invf = np.power(f(10000.0), (np.arange(half, dtype=f) / f(half)).astype(f)).astype(f)
    sinu = (pos[:, None] * invf[None, :]).astype(f)  # [n, half]
    c64 = np.cos(sinu.astype(np.float64)).T  # [half, n]
    s64 = np.sin(sinu.astype(np.float64)).T
    cos_t = np.ascontiguousarray(np.concatenate([c64, c64], axis=0), dtype=f)
    sin_t = np.ascontiguousarray(np.concatenate([-s64, s64], axis=0), dtype=f)
    # q side absorbs the 1/MAX_SEQ of qk/MAX_SEQ
    gbq = np.ascontiguousarray(
        np.stack([gamma[0] / MAX_SEQ, beta[0] / MAX_SEQ], axis=1), dtype=f
    )  # [128, 2]
    gbk = np.ascontiguousarray(np.stack([gamma[1], beta[1]], axis=1), dtype=f)
    ub = np.ascontiguousarray(uv_b_eff[:E].reshape(16, P).T)  # [128, 16]
    vb = np.ascontiguousarray(uv_b_eff[E : 2 * E].reshape(1, E))
    bb = np.ascontiguousarray(uv_b_eff[2 * E :].reshape(P, 1))
    ob = np.ascontiguousarray(o_b.reshape(1, HIDDEN).astype(f))
    shared = {
        "uv_wT": uv_wT,
        "o_wT": o_wT,
        "biasT": biasT_dev,
        "cos": cos_t,
        "sin": sin_t,
        "ub": ub,
        "vb": vb,
        "bb": bb,
        "ob": ob,
        "gbq": gbq,
        "gbk": gbk,
        "ones": np.ones((1, P), dtype=f),
    }
    in_maps = []
    for ci in range(N_CORES):
        m = dict(shared)
        m["x"] = np.ascontiguousarray(
            x[ci * B_LOC : (ci + 1) * B_LOC].reshape(NLOC, HIDDEN)
        )
        in_maps.append(m)
    return in_maps


def kernel(x, ln_w, ln_b, uv_w, uv_b, gamma, beta, o_w, o_b, w_rel):
    from concourse.bass_utils import run_bass_kernel_spmd

    if "nc" not in _CACHE:
        _CACHE["nc"] = _build()
    nc = _CACHE["nc"]
    in_maps = _host_prep(x, ln_w, ln_b, uv_w, uv_b, gamma, beta, o_w, o_b, w_rel)
    res = run_bass_kernel_spmd(nc, in_maps, core_ids=list(range(N_CORES)))
    out = np.concatenate(
        [res.results[ci]["out"].reshape(B_LOC, MAX_SEQ, HIDDEN) for ci in range(N_CORES)],
        axis=0,
    )
    return out.astype(np.float32)


# revision 11
# speedup vs baseline: 1.1033x; 1.1033x over previous
"""GateAttentionUnit Trainium2 kernel.

Full inputs in, full output out. Data-parallel over batch: 16 batches
split 2-per-core across 8 NeuronCores; each core runs an identical NEFF
(SPMD) on its own x slice with replicated weights.

Device math per core (2 sequential batches of [512, 1024]):
  LN (ln affine folded into uv weights on host) -> xn, PE-transposed to
  xnT [d,n] -> uv projection via fp32r matmuls (base head / v natural /
  u transposed orientations) -> silu -> rope'd q,k (host sin/cos
  tables) -> qkT -> kernelT = relu(qkT + biasT)^2 (toeplitz bias table
  from host) -> kvT -> g = u * kv -> out proj + bias + shortcut.
"""

import sys

if "/opt/trn_rl_repo" not in sys.path:
    sys.path.insert(0, "/opt/trn_rl_repo")

import numpy as np

MAX_SEQ = 512
HIDDEN = 1024
E = 2048
S = 128
EPS = 1e-5
N_CORES = 8
B = 16
B_LOC = B // N_CORES  # 2 batches per core
P = 128
NLOC = B_LOC * MAX_SEQ  # 1024 rows per core

_CACHE = {}


class _Ctx:
    pass


def _phase0_ln(c, b):
    """LayerNorm batch b and transpose into c.xnT [d-part, n-free]."""
    import concourse.mybir as mybir

    nc = c.nc
    f32 = mybir.dt.float32
    bf16 = mybir.dt.bfloat16
    Alu = mybir.AluOpType
    Act = mybir.ActivationFunctionType
    r0 = b * MAX_SEQ
    with c.tc.tile_pool(name="psA", bufs=4, space="PSUM") as psA:
        for nt in range(4):
            xt = c.x_pool.tile([P, HIDDEN], f32, tag="xt")
            nc.sync.dma_start(xt[:], c.x_ap[r0 + nt * P : r0 + (nt + 1) * P, :])
            s1 = c.stats.tile([P, 1], f32, tag="s1")
            nc.vector.tensor_reduce(s1[:], xt[:], mybir.AxisListType.X, Alu.add)
            sq = c.x_pool.tile([P, HIDDEN], f32, tag="sq")
            s2 = c.stats.tile([P, 1], f32, tag="s2")
            nc.scalar.activation(sq[:], xt[:], Act.Square, accum_out=s2[:])
            mu = c.stats.tile([P, 1], f32, tag="mu")
            nc.vector.tensor_scalar_mul(mu[:], s1[:], 1.0 / HIDDEN)
            m2 = c.stats.tile([P, 1], f32, tag="m2")
            nc.vector.tensor_scalar_mul(m2[:], s2[:], 1.0 / HIDDEN)
            # var = m2 - mu^2
            var = c.stats.tile([P, 1], f32, tag="var")
            mu2 = c.stats.tile([P, 1], f32, tag="mu2")
            nc.vector.tensor_mul(mu2[:], mu[:], mu[:])
            nc.vector.tensor_sub(var[:], m2[:], mu2[:])
            sd = c.stats.tile([P, 1], f32, tag="sd")
            nc.scalar.activation(sd[:], var[:], Act.Sqrt, bias=c.eps_sb[:])
            rstd = c.stats.tile([P, 1], f32, tag="rstd")
            nc.vector.reciprocal(rstd[:], sd[:])
            nmu = c.stats.tile([P, 1], f32, tag="nmu")
            nc.vector.scalar_tensor_tensor(
                nmu[:], mu[:], -1.0, rstd[:], Alu.mult, Alu.mult
            )  # (-mu) * rstd
            xn = c.x_pool.tile([P, HIDDEN], f32, tag="sq")
            nc.vector.tensor_scalar(xn[:], xt[:], rstd[:], nmu[:], Alu.mult, Alu.add)
            for dt_i in range(8):
                ps = psA.tile([P, P], f32, tag="tp")
                nc.tensor.transpose(ps[:], xn[:, dt_i * P : (dt_i + 1) * P], c.ident[:])
                nc.vector.tensor_copy(c.xnT[:, dt_i, nt * P : (nt + 1) * P], ps[:])


def _phase1_kernelT(c):
    """base head -> silu -> scale-offset -> rope -> qkT -> kernelT."""
    import concourse.mybir as mybir

    nc = c.nc
    f32 = mybir.dt.float32
    bf16 = mybir.dt.bfloat16
    Alu = mybir.AluOpType
    Act = mybir.ActivationFunctionType
    with c.tc.tile_pool(name="psB", bufs=2, space="PSUM") as psB:
        wb = c.wbase_pool.tile([P, 8, S], bf16, tag="wb")
        nc.sync.dma_start(
            wb[:], c.uvw[:, 2 * E : 2 * E + S].rearrange("(do di) f -> di do f", di=P)
        )
        bps = psB.tile([P, MAX_SEQ], f32, tag="bps")
        for k in range(8):
            nc.tensor.matmul(
                bps[:], (wb[:, k, :]), (c.xnT[:, k, :]),
                start=(k == 0), stop=(k == 7),
            )
        base_sb = c.qk1_pool.tile([P, MAX_SEQ], f32, tag="base")
        nc.scalar.activation(base_sb[:], bps[:], Act.Silu, bias=c.bb_sb[:, 0:1])
        q_ro = c.qk1_pool.tile([P, MAX_SEQ], bf16, tag="q_ro")
        k_ro = c.qk1_pool.tile([P, MAX_SEQ], bf16, tag="k_ro")
        for gb_sb, ro in ((c.gbq_sb, q_ro), (c.gbk_sb, k_ro)):
            pre = c.qk_pool.tile([P, MAX_SEQ], f32, tag="pre")
            nc.vector.tensor_scalar(
                pre[:], base_sb[:], gb_sb[:, 0:1], gb_sb[:, 1:2], Alu.mult, Alu.add
            )
            sw = c.qk_pool.tile([P, MAX_SEQ], f32, tag="sw")
            nc.sync.dma_start(sw[0:64, :], pre[64:128, :])
            nc.sync.dma_start(sw[64:128, :], pre[0:64, :])
            tmp = c.qk_pool.tile([P, MAX_SEQ], f32, tag="rtmp")
            nc.vector.tensor_mul(tmp[:], sw[:], c.sin_sb[:])
            nc.vector.tensor_mul(ro[:], pre[:], c.cos_sb[:])
            nc.vector.tensor_add(ro[:], ro[:], tmp[:])
        kernelT = c.kern_pool.tile([P, 4, MAX_SEQ], bf16, tag="kernelT")
        for mt in range(4):
            qps = psB.tile([P, MAX_SEQ], f32, tag="qps")
            nc.tensor.matmul(
                qps[:], (k_ro[:, mt * P : (mt + 1) * P]), (q_ro[:]),
                start=True, stop=True,
            )
            t = c.qk_pool.tile([P, MAX_SEQ], f32, tag="kt_tmp")
            nc.vector.tensor_add(t[:], qps[:], c.bias_sb[:, mt, :])
            # relu(t)^2 == max(t,0)*t
            nc.vector.scalar_tensor_tensor(
                kernelT[:, mt, :], t[:], 0.0, t[:], Alu.max, Alu.mult
            )
        c.kernelT = kernelT


def _phase2_gated(c):
    """v chunks, u tiles, kvT, g = u * kv into c.gT."""
    import concourse.mybir as mybir

    nc = c.nc
    f32 = mybir.dt.float32
    bf16 = mybir.dt.bfloat16
    Act = mybir.ActivationFunctionType
    with c.tc.tile_pool(name="psC", bufs=2, space="PSUM") as psC:
        for ec in range(4):
            wv = c.wv_pool.tile([P, 8, 512], bf16, tag="wv")
            nc.sync.dma_start(
                wv[:],
                c.uvw[:, E + ec * 512 : E + (ec + 1) * 512].rearrange(
                    "(do di) f -> di do f", di=P
                ),
            )
            vch = c.v_pool.tile([P, 4, 512], bf16, tag="vch")
            for nt in range(4):
                vps = psC.tile([P, 512], f32, tag="vps")
                for k in range(8):
                    nc.tensor.matmul(
                        vps[:], (c.xnT[:, k, nt * P : (nt + 1) * P]),
                        (wv[:, k, :]),
                        start=(k == 0), stop=False,
                    )
                nc.tensor.matmul(
                    vps[:], (c.ones_sb[:]),
                    (c.vb_sb[:, ec * 512 : (ec + 1) * 512]),
                    start=False, stop=True,
                )
                nc.scalar.activation(vch[:, nt, :], vps[:], Act.Silu)
            for et in range(4):
                ei = ec * 4 + et
                wu = c.wu_pool.tile([P, 8, P], bf16, tag="wu")
                nc.sync.dma_start(
                    wu[:],
                    c.uvw[:, ei * P : (ei + 1) * P].rearrange(
                        "(do di) f -> di do f", di=P
                    ),
                )
                ups = psC.tile([P, MAX_SEQ], f32, tag="ups")
                for k in range(8):
                    nc.tensor.matmul(
                        ups[:], (wu[:, k, :]), (c.xnT[:, k, :]),
                        start=(k == 0), stop=(k == 7),
                    )
                ut = c.u_pool.tile([P, MAX_SEQ], f32, tag="ut")
                nc.scalar.activation(
                    ut[:], ups[:], Act.Silu, bias=c.ub_sb[:, ei : ei + 1]
                )
                kps = psC.tile([P, MAX_SEQ], f32, tag="kps")
                for mt in range(4):
                    nc.tensor.matmul(
                        kps[:], (vch[:, mt, et * P : (et + 1) * P]),
                        (c.kernelT[:, mt, :]),
                        start=(mt == 0), stop=(mt == 3),
                    )
                nc.vector.tensor_mul(c.gT[:, ei, :], ut[:], kps[:])


def _phase3_out(c, b):
    """out = gT.T @ o_wT + o_b + shortcut, DMA to DRAM."""
    import concourse.mybir as mybir

    nc = c.nc
    f32 = mybir.dt.float32
    bf16 = mybir.dt.bfloat16
    r0 = b * MAX_SEQ
    with c.tc.tile_pool(name="psD", bufs=8, space="PSUM") as psD:
        accs = [psD.tile([P, 512], f32, tag="acc", name=f"acc{i}") for i in range(8)]
        for et in range(16):
            wo = c.wo_pool.tile([P, 2, 512], bf16, tag="wo")
            nc.sync.dma_start(
                wo[:],
                c.owt[et * P : (et + 1) * P, :].rearrange("p (dc f) -> p dc f", dc=2),
            )
            for nt in range(4):
                for dc in range(2):
                    nc.tensor.matmul(
                        accs[nt * 2 + dc][:],
                        (c.gT[:, et, nt * P : (nt + 1) * P]),
                        (wo[:, dc, :]),
                        start=(et == 0), stop=False,
                    )
        for nt in range(4):
            for dc in range(2):
                nc.tensor.matmul(
                    accs[nt * 2 + dc][:], (c.ones_sb[:]),
                    (c.ob_sb[:, dc * 512 : (dc + 1) * 512]),
                    start=False, stop=True,
                )
                osb = c.out_pool.tile([P, 512], f32, tag="osb")
                xs = c.xs_pool.tile([P, 512], f32, tag="xs")
                nc.sync.dma_start(
                    xs[:],
                    c.x_ap[r0 + nt * P : r0 + (nt + 1) * P, dc * 512 : (dc + 1) * 512],
                )
                nc.vector.tensor_add(osb[:], accs[nt * 2 + dc][:], xs[:])
                nc.sync.dma_start(
                    c.out_ap[
                        r0 + nt * P : r0 + (nt + 1) * P, dc * 512 : (dc + 1) * 512
                    ],
                    osb[:],
                )


def _build():
    import concourse.mybir as mybir
    import concourse.tile as tile
    from concourse import bacc
    from concourse.masks import make_identity

    f32 = mybir.dt.float32
    bf16 = mybir.dt.bfloat16

    nc = bacc.Bacc("TRN2", target_bir_lowering=False, debug=False,
                   num_devices=N_CORES)

    x_d = nc.dram_tensor("x", [NLOC, HIDDEN], f32, kind="ExternalInput")
    uvw_d = nc.dram_tensor("uv_wT", [HIDDEN, 2 * E + S], bf16, kind="ExternalInput")
    owt_d = nc.dram_tensor("o_wT", [E, HIDDEN], bf16, kind="ExternalInput")
    bias_d = nc.dram_tensor("biasT", [P, 4, MAX_SEQ], f32, kind="ExternalInput")
    cos_d = nc.dram_tensor("cos", [P, MAX_SEQ], f32, kind="ExternalInput")
    sin_d = nc.dram_tensor("sin", [P, MAX_SEQ], f32, kind="ExternalInput")
    ub_d = nc.dram_tensor("ub", [P, 16], f32, kind="ExternalInput")
    vb_d = nc.dram_tensor("vb", [1, E], bf16, kind="ExternalInput")
    bb_d = nc.dram_tensor("bb", [P, 1], f32, kind="ExternalInput")
    ob_d = nc.dram_tensor("ob", [1, HIDDEN], bf16, kind="ExternalInput")
    gbq_d = nc.dram_tensor("gbq", [P, 2], f32, kind="ExternalInput")
    gbk_d = nc.dram_tensor("gbk", [P, 2], f32, kind="ExternalInput")
    ones_d = nc.dram_tensor("ones", [1, P], bf16, kind="ExternalInput")
    out_d = nc.dram_tensor("out", [NLOC, HIDDEN], f32, kind="ExternalOutput")

    c = _Ctx()
    c.nc = nc
    c.x_ap = x_d.ap()
    c.out_ap = out_d.ap()
    c.uvw = uvw_d.ap()
    c.owt = owt_d.ap()

    with tile.TileContext(nc) as tc:
        c.tc = tc
        with (
            tc.tile_pool(name="const", bufs=1) as const,
            tc.tile_pool(name="xt", bufs=2) as x_pool,
            tc.tile_pool(name="stats", bufs=4) as stats,
            tc.tile_pool(name="xnT", bufs=1) as xnT_pool,
            tc.tile_pool(name="wbase", bufs=1) as wbase_pool,
            tc.tile_pool(name="qk1", bufs=1) as qk1_pool,
            tc.tile_pool(name="qk2", bufs=2) as qk_pool,
            tc.tile_pool(name="kern", bufs=1) as kern_pool,
            tc.tile_pool(name="wv", bufs=2) as wv_pool,
            tc.tile_pool(name="vch", bufs=2) as v_pool,
            tc.tile_pool(name="wu", bufs=2) as wu_pool,
            tc.tile_pool(name="ut", bufs=3) as u_pool,
            tc.tile_pool(name="gT", bufs=1) as gT_pool,
            tc.tile_pool(name="wo", bufs=2) as wo_pool,
            tc.tile_pool(name="outs", bufs=2) as out_pool,
            tc.tile_pool(name="xs", bufs=2) as xs_pool,
        ):
            c.x_pool = x_pool
            c.stats = stats
            c.wbase_pool = wbase_pool
            c.qk1_pool = qk1_pool
            c.qk_pool = qk_pool
            c.kern_pool = kern_pool
            c.wv_pool = wv_pool
            c.v_pool = v_pool
            c.wu_pool = wu_pool
            c.u_pool = u_pool
            c.wo_pool = wo_pool
            c.out_pool = out_pool
            c.xs_pool = xs_pool

            c.ident = const.tile([P, P], f32)
            make_identity(nc, c.ident[:])
            c.cos_sb = const.tile([P, MAX_SEQ], f32)
            nc.sync.dma_start(c.cos_sb[:], cos_d.ap())
            c.sin_sb = const.tile([P, MAX_SEQ], f32)
            nc.sync.dma_start(c.sin_sb[:], sin_d.ap())
            c.bias_sb = const.tile([P, 4, MAX_SEQ], f32)
            nc.sync.dma_start(c.bias_sb[:], bias_d.ap())
            c.ub_sb = const.tile([P, 16], f32)
            nc.sync.dma_start(c.ub_sb[:], ub_d.ap())
            c.vb_sb = const.tile([1, E], bf16)
            nc.sync.dma_start(c.vb_sb[:], vb_d.ap())
            c.bb_sb = const.tile([P, 1], f32)
            nc.sync.dma_start(c.bb_sb[:], bb_d.ap())
            c.ob_sb = const.tile([1, HIDDEN], bf16)
            nc.sync.dma_start(c.ob_sb[:], ob_d.ap())
            c.gbq_sb = const.tile([P, 2], f32)
            nc.sync.dma_start(c.gbq_sb[:], gbq_d.ap())
            c.gbk_sb = const.tile([P, 2], f32)
            nc.sync.dma_start(c.gbk_sb[:], gbk_d.ap())
            c.ones_sb = const.tile([1, P], bf16)
            nc.sync.dma_start(c.ones_sb[:], ones_d.ap())
            c.eps_sb = const.tile([P, 1], f32)
            nc.vector.memset(c.eps_sb[:], EPS)

            for b in range(B_LOC):
                c.xnT = xnT_pool.tile([P, 8, MAX_SEQ], bf16, tag="xnT", name="xnT")
                _phase0_ln(c, b)
                _phase1_kernelT(c)
                c.gT = gT_pool.tile([P, 16, MAX_SEQ], bf16, tag="gT", name="gT")
                _phase2_gated(c)
                _phase3_out(c, b)

    nc.compile()
    return nc


def _host_prep(x, ln_w, ln_b, uv_w, uv_b, gamma, beta, o_w, o_b, w_rel):
    import ml_dtypes

    f = np.float32
    bf = ml_dtypes.bfloat16
    x = np.ascontiguousarray(x, dtype=f)
    # fold LN affine into the uv projection (exact algebra)
    uv_wT = np.ascontiguousarray((uv_w * ln_w[None, :]).T.astype(bf))
    uv_b_eff = (
        uv_b.astype(np.float64) + uv_w.astype(np.float64) @ ln_b.astype(np.float64)
    ).astype(f)
    o_wT = np.ascontiguousarray(o_w.T.astype(bf))
    # toeplitz bias, transposed orientation: biasT[m, n] = bias[n, m] = w_rel[511 + m - n]
    idx = (MAX_SEQ - 1) + np.arange(MAX_SEQ)[:, None] - np.arange(MAX_SEQ)[None, :]
    biasT = w_rel[idx].astype(f)
    biasT_dev = np.ascontiguousarray(biasT.reshape(4, P, MAX_SEQ).transpose(1, 0, 2))
    # rope tables (match reference: fp32 sinusoid, accurate sin/cos)
    pos = np.arange(MAX_SEQ, dtype=f)
    half = S // 2
    invf = np.power(f(10000.0), (np.arange(half, dtype=f) / f(half)).astype(f)).astype(f)
    sinu = (pos[:, None] * invf[None, :]).astype(f)  # [n, half]
    c64 = np.cos(sinu.astype(np.float64)).T  # [half, n]
    s64 = np.sin(sinu.astype(np.float64)).T
    cos_t = np.ascontiguousarray(np.concatenate([c64, c64], axis=0), dtype=f)
    sin_t = np.ascontiguousarray(np.concatenate([-s64, s64], axis=0), dtype=f)
    # q side absorbs the 1/MAX_SEQ of qk/MAX_SEQ
    gbq = np.ascontiguousarray(
        np.stack([gamma[0] / MAX_SEQ, beta[0] / MAX_SEQ], axis=1), dtype=f
    )  # [128, 2]
    gbk = np.ascontiguousarray(np.stack([gamma[1], beta[1]], axis=1), dtype=f)
    ub = np.ascontiguousarray(uv_b_eff[:E].reshape(16, P).T)  # [128, 16]
    vb = np.ascontiguousarray(uv_b_eff[E : 2 * E].reshape(1, E).astype(bf))
    bb = np.ascontiguousarray(uv_b_eff[2 * E :].reshape(P, 1))
    ob = np.ascontiguousarray(o_b.reshape(1, HIDDEN).astype(bf))
    shared = {
        "uv_wT": uv_wT,
        "o_wT": o_wT,
        "biasT": biasT_dev,
        "cos": cos_t,
        "sin": sin_t,
        "ub": ub,
        "vb": vb,
        "bb": bb,
        "ob": ob,
        "gbq": gbq,
        "gbk": gbk,
        "ones": np.ones((1, P), dtype=bf),
    }
    in_maps = []
    for ci in range(N_CORES):
        m = dict(shared)
        m["x"] = np.ascontiguousarray(
            x[ci * B_LOC : (ci + 1) * B_LOC].reshape(NLOC, HIDDEN)
        )
        in_maps.append(m)
    return in_maps


def kernel(x, ln_w, ln_b, uv_w, uv_b, gamma, beta, o_w, o_b, w_rel):
    from concourse.bass_utils import run_bass_kernel_spmd

    if "nc" not in _CACHE:
        _CACHE["nc"] = _build()
    nc = _CACHE["nc"]
    in_maps = _host_prep(x, ln_w, ln_b, uv_w, uv_b, gamma, beta, o_w, o_b, w_rel)
    res = run_bass_kernel_spmd(nc, in_maps, core_ids=list(range(N_CORES)))
    out = np.concatenate(
        [res.results[ci]["out"].reshape(B_LOC, MAX_SEQ, HIDDEN) for ci in range(N_CORES)],
        axis=0,
    )
    return out.astype(np.float32)


# revision 13
# speedup vs baseline: 1.2268x; 1.1119x over previous
"""GateAttentionUnit Trainium2 kernel.

Full inputs in, full output out. Data-parallel over batch: 16 batches
split 2-per-core across 8 NeuronCores; each core runs an identical NEFF
(SPMD) on its own x slice with replicated weights.

Device math per core (2 pipelined batches of [512, 1024]):
  LN (ln affine folded into uv weights on host) -> xn (bf16),
  PE-transposed to xnT [d,n] -> uv projection via bf16 matmuls (base
  head / v natural / u transposed orientations) -> silu -> rope'd q,k
  (host sin/cos tables) -> qkT -> kernelT = relu(qkT + biasT)^2
  (toeplitz bias from host) -> kvT -> g = u * kv -> out proj + bias +
  shortcut.  One global 8-bank PSUM layout: acc0-3 output accumulators
  + w1-w4 working banks, so phases of consecutive batches overlap.
"""

import sys

if "/opt/trn_rl_repo" not in sys.path:
    sys.path.insert(0, "/opt/trn_rl_repo")

import numpy as np

MAX_SEQ = 512
HIDDEN = 1024
E = 2048
S = 128
EPS = 1e-5
N_CORES = 8
B = 16
B_LOC = B // N_CORES  # 2 batches per core
P = 128
NLOC = B_LOC * MAX_SEQ  # 1024 rows per core

_CACHE = {}


class _Ctx:
    pass


def _phase0_ln(c, b):
    """LayerNorm batch b and transpose into c.xnT [d-part, n-free] (bf16)."""
    import concourse.mybir as mybir

    nc = c.nc
    f32 = mybir.dt.float32
    bf16 = mybir.dt.bfloat16
    Alu = mybir.AluOpType
    Act = mybir.ActivationFunctionType
    r0 = b * MAX_SEQ
    for nt in range(4):
        xt = c.x_pool.tile([P, HIDDEN], f32, tag="xt")
        nc.sync.dma_start(xt[:], c.x_ap[r0 + nt * P : r0 + (nt + 1) * P, :])
        s1 = c.stats.tile([P, 1], f32, tag="s1")
        nc.vector.tensor_reduce(s1[:], xt[:], mybir.AxisListType.X, Alu.add)
        sq = c.x_pool.tile([P, HIDDEN], f32, tag="sq")
        s2 = c.stats.tile([P, 1], f32, tag="s2")
        nc.scalar.activation(sq[:], xt[:], Act.Square, accum_out=s2[:])
        mu = c.stats.tile([P, 1], f32, tag="mu")
        nc.vector.tensor_scalar_mul(mu[:], s1[:], 1.0 / HIDDEN)
        m2 = c.stats.tile([P, 1], f32, tag="m2")
        nc.vector.tensor_scalar_mul(m2[:], s2[:], 1.0 / HIDDEN)
        # var = m2 - mu^2
        var = c.stats.tile([P, 1], f32, tag="var")
        mu2 = c.stats.tile([P, 1], f32, tag="mu2")
        nc.vector.tensor_mul(mu2[:], mu[:], mu[:])
        nc.vector.tensor_sub(var[:], m2[:], mu2[:])
        sd = c.stats.tile([P, 1], f32, tag="sd")
        nc.scalar.activation(sd[:], var[:], Act.Sqrt, bias=c.eps_sb[:])
        rstd = c.stats.tile([P, 1], f32, tag="rstd")
        nc.vector.reciprocal(rstd[:], sd[:])
        nmu = c.stats.tile([P, 1], f32, tag="nmu")
        nc.vector.scalar_tensor_tensor(
            nmu[:], mu[:], -1.0, rstd[:], Alu.mult, Alu.mult
        )  # (-mu) * rstd
        xn = c.x_pool.tile([P, HIDDEN], bf16, tag="xn")
        nc.vector.tensor_scalar(xn[:], xt[:], rstd[:], nmu[:], Alu.mult, Alu.add)
        for dt_i in range(8):
            ps = c.ps.tile([P, P], bf16, tag=f"w{(nt * 8 + dt_i) % 2 + 1}", name="tp")
            nc.tensor.transpose(ps[:], xn[:, dt_i * P : (dt_i + 1) * P], c.ident[:])
            nc.vector.tensor_copy(c.xnT[:, dt_i, nt * P : (nt + 1) * P], ps[:])


def _phase1_kernelT(c):
    """base head -> silu -> scale-offset -> rope -> qkT -> kernelT."""
    import concourse.mybir as mybir

    nc = c.nc
    f32 = mybir.dt.float32
    bf16 = mybir.dt.bfloat16
    Alu = mybir.AluOpType
    Act = mybir.ActivationFunctionType
    wb = c.wbase_pool.tile([P, 8, S], bf16, tag="wb")
    nc.sync.dma_start(
        wb[:], c.uvw[:, 2 * E : 2 * E + S].rearrange("(do di) f -> di do f", di=P)
    )
    bps = c.ps.tile([P, MAX_SEQ], f32, tag="w3", name="bps")
    for k in range(8):
        nc.tensor.matmul(
            bps[:], wb[:, k, :], c.xnT[:, k, :], start=(k == 0), stop=(k == 7)
        )
    base_sb = c.qk1_pool.tile([P, MAX_SEQ], f32, tag="base")
    nc.scalar.activation(base_sb[:], bps[:], Act.Silu, bias=c.bb_sb[:, 0:1])
    q_ro = c.qk1_pool.tile([P, MAX_SEQ], bf16, tag="q_ro")
    k_ro = c.qk1_pool.tile([P, MAX_SEQ], bf16, tag="k_ro")
    for gb_sb, ro in ((c.gbq_sb, q_ro), (c.gbk_sb, k_ro)):
        pre = c.qk_pool.tile([P, MAX_SEQ], f32, tag="pre")
        nc.vector.tensor_scalar(
            pre[:], base_sb[:], gb_sb[:, 0:1], gb_sb[:, 1:2], Alu.mult, Alu.add
        )
        sw = c.qk_pool.tile([P, MAX_SEQ], f32, tag="sw")
        nc.sync.dma_start(sw[0:64, :], pre[64:128, :])
        nc.sync.dma_start(sw[64:128, :], pre[0:64, :])
        tmp = c.qk_pool.tile([P, MAX_SEQ], f32, tag="rtmp")
        nc.vector.tensor_mul(tmp[:], sw[:], c.sin_sb[:])
        ro32 = c.qk_pool.tile([P, MAX_SEQ], f32, tag="ro32")
        nc.vector.tensor_mul(ro32[:], pre[:], c.cos_sb[:])
        nc.vector.tensor_add(ro[:], ro32[:], tmp[:])
    kernelT = c.kern_pool.tile([P, 4, MAX_SEQ], bf16, tag="kernelT")
    for mt in range(4):
        qps = c.ps.tile([P, MAX_SEQ], f32, tag=f"w{mt % 2 + 1}", name="qps")
        nc.tensor.matmul(
            qps[:], k_ro[:, mt * P : (mt + 1) * P], q_ro[:], start=True, stop=True
        )
        t = c.qk_pool.tile([P, MAX_SEQ], f32, tag="kt_tmp")
        nc.vector.tensor_add(t[:], qps[:], c.bias_sb[:, mt, :])
        # relu(t)^2 == max(t,0)*t
        nc.vector.scalar_tensor_tensor(
            kernelT[:, mt, :], t[:], 0.0, t[:], Alu.max, Alu.mult
        )
    c.kernelT = kernelT


def _phase2_gated(c):
    """v chunks, u tiles, kvT, g = u * kv into c.gT."""
    import concourse.mybir as mybir

    nc = c.nc
    f32 = mybir.dt.float32
    bf16 = mybir.dt.bfloat16
    Act = mybir.ActivationFunctionType
    for ec in range(4):
        wv = c.wv_pool.tile([P, 8, 512], bf16, tag="wv")
        nc.sync.dma_start(
            wv[:],
            c.uvw[:, E + ec * 512 : E + (ec + 1) * 512].rearrange(
                "(do di) f -> di do f", di=P
            ),
        )
        vch = c.v_pool.tile([P, 4, 512], bf16, tag="vch")
        for nt in range(4):
            vps = c.ps.tile([P, 512], f32, tag=f"w{nt % 2 + 1}", name="vps")
            for k in range(8):
                nc.tensor.matmul(
                    vps[:], c.xnT[:, k, nt * P : (nt + 1) * P], wv[:, k, :],
                    start=(k == 0), stop=False,
                )
            nc.tensor.matmul(
                vps[:], c.ones_sb[:], c.vb_sb[:, ec * 512 : (ec + 1) * 512],
                start=False, stop=True,
            )
            nc.scalar.activation(vch[:, nt, :], vps[:], Act.Silu)
        for et in range(4):
            ei = ec * 4 + et
            wu = c.wu_pool.tile([P, 8, P], bf16, tag="wu")
            nc.sync.dma_start(
                wu[:],
                c.uvw[:, ei * P : (ei + 1) * P].rearrange("(do di) f -> di do f", di=P),
            )
            ups = c.ps.tile([P, MAX_SEQ], f32, tag="w3", name="ups")
            for k in range(8):
                nc.tensor.matmul(
                    ups[:], wu[:, k, :], c.xnT[:, k, :], start=(k == 0), stop=(k == 7)
                )
            ut = c.u_pool.tile([P, MAX_SEQ], f32, tag="ut")
            nc.scalar.activation(ut[:], ups[:], Act.Silu, bias=c.ub_sb[:, ei : ei + 1])
            kps = c.ps.tile([P, MAX_SEQ], f32, tag="w4", name="kps")
            for mt in range(4):
                nc.tensor.matmul(
                    kps[:], vch[:, mt, et * P : (et + 1) * P], c.kernelT[:, mt, :],
                    start=(mt == 0), stop=(mt == 3),
                )
            nc.vector.tensor_mul(c.gT[:, ei, :], ut[:], kps[:])


def _phase3_out(c, b):
    """out = gT.T @ o_wT + o_b + shortcut, DMA to DRAM.

    Two half-d sweeps of 4 PSUM accumulators each, so only half of PSUM
    is tied up and the next batch's phases can use the working banks.
    """
    import concourse.mybir as mybir

    nc = c.nc
    f32 = mybir.dt.float32
    bf16 = mybir.dt.bfloat16
    r0 = b * MAX_SEQ
    for dc in range(2):
        accs = [
            c.ps.tile([P, 512], f32, tag=f"acc{i}", name=f"acc{i}") for i in range(4)
        ]
        for et in range(16):
            wo = c.wo_pool.tile([P, 512], bf16, tag="wo")
            nc.sync.dma_start(
                wo[:], c.owt[et * P : (et + 1) * P, dc * 512 : (dc + 1) * 512]
            )
            for nt in range(4):
                nc.tensor.matmul(
                    accs[nt][:], c.gT[:, et, nt * P : (nt + 1) * P], wo[:],
                    start=(et == 0), stop=False,
                )
        for nt in range(4):
            nc.tensor.matmul(
                accs[nt][:], c.ones_sb[:], c.ob_sb[:, dc * 512 : (dc + 1) * 512],
                start=False, stop=True,
            )
            osb = c.out_pool.tile([P, 512], f32, tag="osb")
            xs = c.xs_pool.tile([P, 512], f32, tag="xs")
            nc.sync.dma_start(
                xs[:],
                c.x_ap[r0 + nt * P : r0 + (nt + 1) * P, dc * 512 : (dc + 1) * 512],
            )
            nc.vector.tensor_add(osb[:], accs[nt][:], xs[:])
            nc.sync.dma_start(
                c.out_ap[r0 + nt * P : r0 + (nt + 1) * P, dc * 512 : (dc + 1) * 512],
                osb[:],
            )


def _build():
    import concourse.mybir as mybir
    import concourse.tile as tile
    from concourse import bacc
    from concourse.masks import make_identity

    f32 = mybir.dt.float32
    bf16 = mybir.dt.bfloat16

    nc = bacc.Bacc("TRN2", target_bir_lowering=False, debug=False,
                   num_devices=N_CORES)

    x_d = nc.dram_tensor("x", [NLOC, HIDDEN], f32, kind="ExternalInput")
    uvw_d = nc.dram_tensor("uv_wT", [HIDDEN, 2 * E + S], bf16, kind="ExternalInput")
    owt_d = nc.dram_tensor("o_wT", [E, HIDDEN], bf16, kind="ExternalInput")
    bias_d = nc.dram_tensor("biasT", [P, 4, MAX_SEQ], f32, kind="ExternalInput")
    cos_d = nc.dram_tensor("cos", [P, MAX_SEQ], f32, kind="ExternalInput")
    sin_d = nc.dram_tensor("sin", [P, MAX_SEQ], f32, kind="ExternalInput")
    ub_d = nc.dram_tensor("ub", [P, 16], f32, kind="ExternalInput")
    vb_d = nc.dram_tensor("vb", [1, E], bf16, kind="ExternalInput")
    bb_d = nc.dram_tensor("bb", [P, 1], f32, kind="ExternalInput")
    ob_d = nc.dram_tensor("ob", [1, HIDDEN], bf16, kind="ExternalInput")
    gbq_d = nc.dram_tensor("gbq", [P, 2], f32, kind="ExternalInput")
    gbk_d = nc.dram_tensor("gbk", [P, 2], f32, kind="ExternalInput")
    ones_d = nc.dram_tensor("ones", [1, P], bf16, kind="ExternalInput")
    out_d = nc.dram_tensor("out", [NLOC, HIDDEN], f32, kind="ExternalOutput")

    c = _Ctx()
    c.nc = nc
    c.x_ap = x_d.ap()
    c.out_ap = out_d.ap()
    c.uvw = uvw_d.ap()
    c.owt = owt_d.ap()

    with tile.TileContext(nc) as tc:
        c.tc = tc
        with (
            tc.tile_pool(name="const", bufs=1) as const,
            tc.tile_pool(name="xt", bufs=2) as x_pool,
            tc.tile_pool(name="stats", bufs=4) as stats,
            tc.tile_pool(name="xnT", bufs=2) as xnT_pool,
            tc.tile_pool(name="wbase", bufs=2) as wbase_pool,
            tc.tile_pool(name="qk1", bufs=2) as qk1_pool,
            tc.tile_pool(name="qk2", bufs=2) as qk_pool,
            tc.tile_pool(name="kern", bufs=2) as kern_pool,
            tc.tile_pool(name="wv", bufs=2) as wv_pool,
            tc.tile_pool(name="vch", bufs=2) as v_pool,
            tc.tile_pool(name="wu", bufs=3) as wu_pool,
            tc.tile_pool(name="ut", bufs=3) as u_pool,
            tc.tile_pool(name="gT", bufs=2) as gT_pool,
            tc.tile_pool(name="wo", bufs=3) as wo_pool,
            tc.tile_pool(name="outs", bufs=3) as out_pool,
            tc.tile_pool(name="xs", bufs=3) as xs_pool,
            tc.tile_pool(name="ps", bufs=1, space="PSUM") as ps,
        ):
            c.x_pool = x_pool
            c.stats = stats
            c.wbase_pool = wbase_pool
            c.qk1_pool = qk1_pool
            c.qk_pool = qk_pool
            c.kern_pool = kern_pool
            c.wv_pool = wv_pool
            c.v_pool = v_pool
            c.wu_pool = wu_pool
            c.u_pool = u_pool
            c.wo_pool = wo_pool
            c.out_pool = out_pool
            c.xs_pool = xs_pool
            c.ps = ps

            c.ident = const.tile([P, P], bf16)
            make_identity(nc, c.ident[:])
            c.cos_sb = const.tile([P, MAX_SEQ], f32)
            nc.sync.dma_start(c.cos_sb[:], cos_d.ap())
            c.sin_sb = const.tile([P, MAX_SEQ], f32)
            nc.sync.dma_start(c.sin_sb[:], sin_d.ap())
            c.bias_sb = const.tile([P, 4, MAX_SEQ], f32)
            nc.sync.dma_start(c.bias_sb[:], bias_d.ap())
            c.ub_sb = const.tile([P, 16], f32)
            nc.sync.dma_start(c.ub_sb[:], ub_d.ap())
            c.vb_sb = const.tile([1, E], bf16)
            nc.sync.dma_start(c.vb_sb[:], vb_d.ap())
            c.bb_sb = const.tile([P, 1], f32)
            nc.sync.dma_start(c.bb_sb[:], bb_d.ap())
            c.ob_sb = const.tile([1, HIDDEN], bf16)
            nc.sync.dma_start(c.ob_sb[:], ob_d.ap())
            c.gbq_sb = const.tile([P, 2], f32)
            nc.sync.dma_start(c.gbq_sb[:], gbq_d.ap())
            c.gbk_sb = const.tile([P, 2], f32)
            nc.sync.dma_start(c.gbk_sb[:], gbk_d.ap())
            c.ones_sb = const.tile([1, P], bf16)
            nc.sync.dma_start(c.ones_sb[:], ones_d.ap())
            c.eps_sb = const.tile([P, 1], f32)
            nc.vector.memset(c.eps_sb[:], EPS)

            for b in range(B_LOC):
                c.xnT = xnT_pool.tile([P, 8, MAX_SEQ], bf16, tag="xnT", name="xnT")
                _phase0_ln(c, b)
                _phase1_kernelT(c)
                c.gT = gT_pool.tile([P, 16, MAX_SEQ], bf16, tag="gT", name="gT")
                _phase2_gated(c)
                _phase3_out(c, b)

    nc.compile()
    return nc


def _host_prep(x, ln_w, ln_b, uv_w, uv_b, gamma, beta, o_w, o_b, w_rel):
    import ml_dtypes

    f = np.float32
    bf = ml_dtypes.bfloat16
    x = np.ascontiguousarray(x, dtype=f)
    # fold LN affine into the uv projection (exact algebra)
    uv_wT = np.ascontiguousarray((uv_w * ln_w[None, :]).T.astype(bf))
    uv_b_eff = (
        uv_b.astype(np.float64) + uv_w.astype(np.float64) @ ln_b.astype(np.float64)
    ).astype(f)
    o_wT = np.ascontiguousarray(o_w.T.astype(bf))
    # toeplitz bias, transposed orientation: biasT[m, n] = bias[n, m] = w_rel[511 + m - n]
    idx = (MAX_SEQ - 1) + np.arange(MAX_SEQ)[:, None] - np.arange(MAX_SEQ)[None, :]
    biasT = w_rel[idx].astype(f)
    biasT_dev = np.ascontiguousarray(biasT.reshape(4, P, MAX_SEQ).transpose(1, 0, 2))
    # rope tables (match reference: fp32 sinusoid, accurate sin/cos)
    pos = np.arange(MAX_SEQ, dtype=f)
    half = S // 2
    invf = np.power(f(10000.0), (np.arange(half, dtype=f) / f(half)).astype(f)).astype(f)
    sinu = (pos[:, None] * invf[None, :]).astype(f)  # [n, half]
    c64 = np.cos(sinu.astype(np.float64)).T  # [half, n]
    s64 = np.sin(sinu.astype(np.float64)).T
    cos_t = np.ascontiguousarray(np.concatenate([c64, c64], axis=0), dtype=f)
    sin_t = np.ascontiguousarray(np.concatenate([-s64, s64], axis=0), dtype=f)
    # q side absorbs the 1/MAX_SEQ of qk/MAX_SEQ
    gbq = np.ascontiguousarray(
        np.stack([gamma[0] / MAX_SEQ, beta[0] / MAX_SEQ], axis=1), dtype=f
    )  # [128, 2]
    gbk = np.ascontiguousarray(np.stack([gamma[1], beta[1]], axis=1), dtype=f)
    ub = np.ascontiguousarray(uv_b_eff[:E].reshape(16, P).T)  # [128, 16]
    vb = np.ascontiguousarray(uv_b_eff[E : 2 * E].reshape(1, E).astype(bf))
    bb = np.ascontiguousarray(uv_b_eff[2 * E :].reshape(P, 1))
    ob = np.ascontiguousarray(o_b.reshape(1, HIDDEN).astype(bf))
    shared = {
        "uv_wT": uv_wT,
        "o_wT": o_wT,
        "biasT": biasT_dev,
        "cos": cos_t,
        "sin": sin_t,
        "ub": ub,
        "vb": vb,
        "bb": bb,
        "ob": ob,
        "gbq": gbq,
        "gbk": gbk,
        "ones": np.ones((1, P), dtype=bf),
    }
    in_maps = []
    for ci in range(N_CORES):
        m = dict(shared)
        m["x"] = np.ascontiguousarray(
            x[ci * B_LOC : (ci + 1) * B_LOC].reshape(NLOC, HIDDEN)
        )
        in_maps.append(m)
    return in_maps


def kernel(x, ln_w, ln_b, uv_w, uv_b, gamma, beta, o_w, o_b, w_rel):
    from concourse.bass_utils import run_bass_kernel_spmd

    if "nc" not in _CACHE:
        _CACHE["nc"] = _build()
    nc = _CACHE["nc"]
    in_maps = _host_prep(x, ln_w, ln_b, uv_w, uv_b, gamma, beta, o_w, o_b, w_rel)
    res = run_bass_kernel_spmd(nc, in_maps, core_ids=list(range(N_CORES)))
    out = np.concatenate(
        [res.results[ci]["out"].reshape(B_LOC, MAX_SEQ, HIDDEN) for ci in range(N_CORES)],
        axis=0,
    )
    return out.astype(np.float32)


# revision 14
# speedup vs baseline: 1.3211x; 1.0768x over previous
"""GateAttentionUnit Trainium2 kernel.

Full inputs in, full output out. Data-parallel over batch: 16 batches
split 2-per-core across 8 NeuronCores; each core runs an identical NEFF
(SPMD) on its own x slice with replicated weights.

Device math per core (2 pipelined batches of [512, 1024]):
  LN (ln affine folded into uv weights on host) -> xn (bf16),
  PE-transposed to xnT [d,n] -> uv projection via bf16 matmuls (base
  head / v natural / u transposed orientations) -> silu -> rope'd q,k
  (host sin/cos tables) -> qkT -> kernelT = relu(qkT + biasT)^2
  (toeplitz bias from host) -> kvT -> g = u * kv -> out proj + bias +
  shortcut.  One global 8-bank PSUM layout: acc0-3 output accumulators
  + w1-w4 working banks, so phases of consecutive batches overlap.
  Weights are host-re-laid-out for contiguous DMA; o_wT stays resident;
  weight DMAs issue on GpSimd, x/activations on Sync.
"""

import sys

if "/opt/trn_rl_repo" not in sys.path:
    sys.path.insert(0, "/opt/trn_rl_repo")

import numpy as np

MAX_SEQ = 512
HIDDEN = 1024
E = 2048
S = 128
EPS = 1e-5
N_CORES = 8
B = 16
B_LOC = B // N_CORES
P = 128
NLOC = B_LOC * MAX_SEQ

# packed f32 const blob layout (free-dim offsets)
CST_COS = 0
CST_SIN = 512
CST_BIAS = 1024          # [4, 512] flattened
CST_UB = 3072            # [16]
CST_BB = 3088            # [1]
CST_GBQ = 3089           # [2]
CST_GBK = 3091           # [2]
CST_W = 3093

_CACHE = {}


class _Ctx:
    pass


def _phase0_ln(c, b):
    """LayerNorm batch b and transpose into c.xnT [d-part, n-free] (bf16)."""
    import concourse.mybir as mybir

    nc = c.nc
    f32 = mybir.dt.float32
    bf16 = mybir.dt.bfloat16
    Alu = mybir.AluOpType
    Act = mybir.ActivationFunctionType
    r0 = b * MAX_SEQ
    c.xts = []
    for nt in range(4):
        xt = c.xres.tile([P, HIDDEN], f32, tag="xt", name=f"xt{b}_{nt}")
        c.xts.append(xt)
        nc.sync.dma_start(xt[:], c.x_ap[r0 + nt * P : r0 + (nt + 1) * P, :])
        s1 = c.stats.tile([P, 1], f32, tag="s1")
        nc.vector.tensor_reduce(s1[:], xt[:], mybir.AxisListType.X, Alu.add)
        sq = c.ln_pool.tile([P, HIDDEN], f32, tag="sq")
        s2 = c.stats.tile([P, 1], f32, tag="s2")
        nc.scalar.activation(sq[:], xt[:], Act.Square, accum_out=s2[:])
        mu = c.stats.tile([P, 1], f32, tag="mu")
        nc.vector.tensor_scalar_mul(mu[:], s1[:], 1.0 / HIDDEN)
        m2 = c.stats.tile([P, 1], f32, tag="m2")
        nc.vector.tensor_scalar_mul(m2[:], s2[:], 1.0 / HIDDEN)
        var = c.stats.tile([P, 1], f32, tag="var")
        mu2 = c.stats.tile([P, 1], f32, tag="mu2")
        nc.vector.tensor_mul(mu2[:], mu[:], mu[:])
        nc.vector.tensor_sub(var[:], m2[:], mu2[:])
        sd = c.stats.tile([P, 1], f32, tag="sd")
        nc.scalar.activation(sd[:], var[:], Act.Sqrt, bias=c.eps_sb[:])
        rstd = c.stats.tile([P, 1], f32, tag="rstd")
        nc.vector.reciprocal(rstd[:], sd[:])
        nmu = c.stats.tile([P, 1], f32, tag="nmu")
        nc.vector.scalar_tensor_tensor(
            nmu[:], mu[:], -1.0, rstd[:], Alu.mult, Alu.mult
        )
        xn = c.ln_pool.tile([P, HIDDEN], bf16, tag="xn")
        nc.vector.tensor_scalar(xn[:], xt[:], rstd[:], nmu[:], Alu.mult, Alu.add)
        for dt_i in range(8):
            ps = c.ps.tile([P, P], bf16, tag=f"w{(nt * 8 + dt_i) % 2 + 1}", name="tp")
            nc.tensor.transpose(ps[:], xn[:, dt_i * P : (dt_i + 1) * P], c.ident[:])
            nc.vector.tensor_copy(c.xnT[:, dt_i, nt * P : (nt + 1) * P], ps[:])


def _phase1_kernelT(c):
    """base head -> silu -> scale-offset -> rope -> qkT -> kernelT."""
    import concourse.mybir as mybir

    nc = c.nc
    f32 = mybir.dt.float32
    bf16 = mybir.dt.bfloat16
    Alu = mybir.AluOpType
    Act = mybir.ActivationFunctionType
    wb = c.wb_pool.tile([P, 8, S], bf16, tag="wb")
    nc.gpsimd.dma_start(wb[:], c.wb_ap)
    bps = c.ps.tile([P, MAX_SEQ], f32, tag="w3", name="bps")
    for k in range(8):
        nc.tensor.matmul(
            bps[:], wb[:, k, :], c.xnT[:, k, :], start=(k == 0), stop=(k == 7)
        )
    base_sb = c.qk1_pool.tile([P, MAX_SEQ], f32, tag="base")
    nc.scalar.activation(base_sb[:], bps[:], Act.Silu, bias=c.bb_sb)
    q_ro = c.qk1_pool.tile([P, MAX_SEQ], bf16, tag="q_ro")
    k_ro = c.qk1_pool.tile([P, MAX_SEQ], bf16, tag="k_ro")
    for gb0, gb1, ro in ((CST_GBQ, CST_GBQ + 1, q_ro), (CST_GBK, CST_GBK + 1, k_ro)):
        pre = c.qk_pool.tile([P, MAX_SEQ], f32, tag="pre")
        nc.vector.tensor_scalar(
            pre[:], base_sb[:],
            c.cst[:, gb0 : gb0 + 1], c.cst[:, gb1 : gb1 + 1],
            Alu.mult, Alu.add,
        )
        sw = c.qk_pool.tile([P, MAX_SEQ], f32, tag="sw")
        nc.sync.dma_start(sw[0:64, :], pre[64:128, :])
        nc.sync.dma_start(sw[64:128, :], pre[0:64, :])
        tmp = c.qk_pool.tile([P, MAX_SEQ], f32, tag="rtmp")
        nc.vector.tensor_mul(tmp[:], sw[:], c.sin_sb)
        ro32 = c.qk_pool.tile([P, MAX_SEQ], f32, tag="ro32")
        nc.vector.tensor_mul(ro32[:], pre[:], c.cos_sb)
        nc.vector.tensor_add(ro[:], ro32[:], tmp[:])
    kernelT = c.kern_pool.tile([P, 4, MAX_SEQ], bf16, tag="kernelT")
    for mt in range(4):
        qps = c.ps.tile([P, MAX_SEQ], f32, tag=f"w{mt % 2 + 1}", name="qps")
        nc.tensor.matmul(
            qps[:], k_ro[:, mt * P : (mt + 1) * P], q_ro[:], start=True, stop=True
        )
        t = c.qk_pool.tile([P, MAX_SEQ], f32, tag="kt_tmp")
        nc.vector.tensor_add(t[:], qps[:], c.bias_sb[:, mt, :])
        nc.vector.scalar_tensor_tensor(
            kernelT[:, mt, :], t[:], 0.0, t[:], Alu.max, Alu.mult
        )
    c.kernelT = kernelT


def _phase2_gated(c):
    """v chunks, u tiles, kvT, g = u * kv into c.gT."""
    import concourse.mybir as mybir

    nc = c.nc
    f32 = mybir.dt.float32
    bf16 = mybir.dt.bfloat16
    Act = mybir.ActivationFunctionType
    for ec in range(4):
        wv = c.wv_pool.tile([P, 8, 512], bf16, tag="wv")
        nc.gpsimd.dma_start(wv[:], c.wv_ap[ec])
        vch = c.v_pool.tile([P, 4, 512], bf16, tag="vch")
        for nt in range(4):
            vps = c.ps.tile([P, 512], f32, tag=f"w{nt % 2 + 1}", name="vps")
            for k in range(8):
                nc.tensor.matmul(
                    vps[:], c.xnT[:, k, nt * P : (nt + 1) * P], wv[:, k, :],
                    start=(k == 0), stop=False,
                )
            nc.tensor.matmul(
                vps[:], c.ones_sb, c.vb_sb[:, ec * 512 : (ec + 1) * 512],
                start=False, stop=True,
            )
            nc.scalar.activation(vch[:, nt, :], vps[:], Act.Silu)
        for eip in range(2):  # ei pairs within chunk
            wu = c.wu_pool.tile([P, 2, 8, P], bf16, tag="wu")
            nc.gpsimd.dma_start(wu[:], c.wu_ap[ec * 2 + eip])
            for sub in range(2):
                ei = ec * 4 + eip * 2 + sub
                et = eip * 2 + sub
                ups = c.ps.tile([P, MAX_SEQ], f32, tag="w3", name="ups")
                for k in range(8):
                    nc.tensor.matmul(
                        ups[:], wu[:, sub, k, :], c.xnT[:, k, :],
                        start=(k == 0), stop=(k == 7),
                    )
                ut = c.u_pool.tile([P, MAX_SEQ], f32, tag="ut")
                nc.scalar.activation(
                    ut[:], ups[:], Act.Silu, bias=c.ub_sb[:, ei : ei + 1]
                )
                kps = c.ps.tile([P, MAX_SEQ], f32, tag="w4", name="kps")
                for mt in range(4):
                    nc.tensor.matmul(
                        kps[:], vch[:, mt, et * P : (et + 1) * P],
                        c.kernelT[:, mt, :],
                        start=(mt == 0), stop=(mt == 3),
                    )
                nc.vector.tensor_mul(c.gT[:, ei, :], ut[:], kps[:])


def _phase3_out(c, b):
    """out = gT.T @ o_wT + o_b + shortcut, two half-d sweeps of 4 accumulators."""
    import concourse.mybir as mybir

    nc = c.nc
    f32 = mybir.dt.float32
    r0 = b * MAX_SEQ
    for dc in range(2):
        accs = [
            c.ps.tile([P, 512], f32, tag=f"acc{i}", name=f"acc{i}") for i in range(4)
        ]
        for et in range(16):
            for nt in range(4):
                nc.tensor.matmul(
                    accs[nt][:], c.gT[:, et, nt * P : (nt + 1) * P],
                    c.wo_sb[:, et, dc * 512 : (dc + 1) * 512],
                    start=(et == 0), stop=False,
                )
        for nt in range(4):
            nc.tensor.matmul(
                accs[nt][:], c.ones_sb, c.ob_sb[:, dc * 512 : (dc + 1) * 512],
                start=False, stop=True,
            )
            osb = c.out_pool.tile([P, 512], f32, tag="osb")
            nc.vector.tensor_add(
                osb[:], accs[nt][:], c.xts[nt][:, dc * 512 : (dc + 1) * 512]
            )
            nc.gpsimd.dma_start(
                c.out_ap[r0 + nt * P : r0 + (nt + 1) * P, dc * 512 : (dc + 1) * 512],
                osb[:],
            )


def _build():
    import concourse.mybir as mybir
    import concourse.tile as tile
    from concourse import bacc
    from concourse.masks import make_identity

    f32 = mybir.dt.float32
    bf16 = mybir.dt.bfloat16

    nc = bacc.Bacc("TRN2", target_bir_lowering=False, debug=False,
                   num_devices=N_CORES)

    x_d = nc.dram_tensor("x", [NLOC, HIDDEN], f32, kind="ExternalInput")
    wu_d = nc.dram_tensor("w_u", [16, P, 8, P], bf16, kind="ExternalInput")
    wv_d = nc.dram_tensor("w_v", [4, P, 8, 512], bf16, kind="ExternalInput")
    wb_d = nc.dram_tensor("w_b", [P, 8, S], bf16, kind="ExternalInput")
    wo_d = nc.dram_tensor("w_o", [16, P, HIDDEN], bf16, kind="ExternalInput")
    cst_d = nc.dram_tensor("cst", [P, CST_W], f32, kind="ExternalInput")
    cstr_d = nc.dram_tensor("cstr", [1, E + HIDDEN + P], bf16, kind="ExternalInput")
    out_d = nc.dram_tensor("out", [NLOC, HIDDEN], f32, kind="ExternalOutput")

    c = _Ctx()
    c.nc = nc
    c.x_ap = x_d.ap()
    c.out_ap = out_d.ap()
    c.wb_ap = wb_d.ap()
    c.wu_ap = [
        wu_d.ap()[2 * i : 2 * i + 2].rearrange("e p do f -> p e do f") for i in range(8)
    ]
    c.wv_ap = [wv_d.ap()[i] for i in range(4)]

    with tile.TileContext(nc) as tc:
        c.tc = tc
        with (
            tc.tile_pool(name="const", bufs=1) as const,
            tc.tile_pool(name="xres", bufs=5) as xres,
            tc.tile_pool(name="ln", bufs=2) as ln_pool,
            tc.tile_pool(name="stats", bufs=4) as stats,
            tc.tile_pool(name="xnT", bufs=2) as xnT_pool,
            tc.tile_pool(name="wb", bufs=2) as wb_pool,
            tc.tile_pool(name="qk1", bufs=1) as qk1_pool,
            tc.tile_pool(name="qk2", bufs=1) as qk_pool,
            tc.tile_pool(name="kern", bufs=2) as kern_pool,
            tc.tile_pool(name="wv", bufs=2) as wv_pool,
            tc.tile_pool(name="vch", bufs=2) as v_pool,
            tc.tile_pool(name="wu", bufs=3) as wu_pool,
            tc.tile_pool(name="ut", bufs=3) as u_pool,
            tc.tile_pool(name="gT", bufs=2) as gT_pool,
            tc.tile_pool(name="outs", bufs=3) as out_pool,
            tc.tile_pool(name="ps", bufs=1, space="PSUM") as ps,
        ):
            c.xres = xres
            c.ln_pool = ln_pool
            c.stats = stats
            c.wb_pool = wb_pool
            c.qk1_pool = qk1_pool
            c.qk_pool = qk_pool
            c.kern_pool = kern_pool
            c.wv_pool = wv_pool
            c.v_pool = v_pool
            c.wu_pool = wu_pool
            c.u_pool = u_pool
            c.out_pool = out_pool
            c.ps = ps

            c.ident = const.tile([P, P], bf16)
            make_identity(nc, c.ident[:])
            c.eps_sb = const.tile([P, 1], f32)
            nc.vector.memset(c.eps_sb[:], EPS)

            cst = const.tile([P, CST_W], f32)
            nc.sync.dma_start(cst[:], cst_d.ap())
            c.cst = cst
            c.cos_sb = cst[:, CST_COS : CST_COS + 512]
            c.sin_sb = cst[:, CST_SIN : CST_SIN + 512]
            c.bias_sb = cst[:, CST_BIAS : CST_BIAS + 2048].rearrange(
                "p (mt n) -> p mt n", mt=4
            )
            c.ub_sb = cst[:, CST_UB : CST_UB + 16]
            c.bb_sb = cst[:, CST_BB : CST_BB + 1]

            cstr = const.tile([1, E + HIDDEN + P], bf16)
            nc.sync.dma_start(cstr[:], cstr_d.ap())
            c.vb_sb = cstr[:, 0:E]
            c.ob_sb = cstr[:, E : E + HIDDEN]
            c.ones_sb = cstr[:, E + HIDDEN : E + HIDDEN + P]

            wo_sb = const.tile([P, 16, HIDDEN], bf16)
            nc.gpsimd.dma_start(wo_sb[:], wo_d.ap().rearrange("e p d -> p e d"))
            c.wo_sb = wo_sb

            for b in range(B_LOC):
                c.xnT = xnT_pool.tile([P, 8, MAX_SEQ], bf16, tag="xnT", name="xnT")
                _phase0_ln(c, b)
                _phase1_kernelT(c)
                c.gT = gT_pool.tile([P, 16, MAX_SEQ], bf16, tag="gT", name="gT")
                _phase2_gated(c)
                _phase3_out(c, b)

    nc.compile()
    return nc


def _host_prep(x, ln_w, ln_b, uv_w, uv_b, gamma, beta, o_w, o_b, w_rel):
    import ml_dtypes

    f = np.float32
    bf = ml_dtypes.bfloat16
    x = np.ascontiguousarray(x, dtype=f)
    # fold LN affine into the uv projection (exact algebra)
    uv_wT = (uv_w * ln_w[None, :]).T.astype(bf)  # [1024, 4224]
    uv_b_eff = (
        uv_b.astype(np.float64) + uv_w.astype(np.float64) @ ln_b.astype(np.float64)
    ).astype(f)
    # weight re-layouts for contiguous DMA
    w_u = np.ascontiguousarray(
        uv_wT[:, :E].reshape(8, P, 16, P).transpose(2, 1, 0, 3)
    )  # [ei, di, do, f]
    w_v = np.ascontiguousarray(
        uv_wT[:, E : 2 * E].reshape(8, P, 4, 512).transpose(2, 1, 0, 3)
    )  # [ec, di, do, f]
    w_b = np.ascontiguousarray(
        uv_wT[:, 2 * E :].reshape(8, P, S).transpose(1, 0, 2)
    )  # [di, do, f]
    w_o = np.ascontiguousarray(o_w.T.astype(bf).reshape(16, P, HIDDEN))  # [et, p, d]
    # toeplitz bias, transposed: biasT[m, n] = bias[n, m] = w_rel[511 + m - n]
    idx = (MAX_SEQ - 1) + np.arange(MAX_SEQ)[:, None] - np.arange(MAX_SEQ)[None, :]
    biasT = w_rel[idx].astype(f)
    biasT_dev = biasT.reshape(4, P, MAX_SEQ).transpose(1, 0, 2)  # [p, mt, n]
    # rope tables (fp32 sinusoid to match reference rounding, accurate sin/cos)
    pos = np.arange(MAX_SEQ, dtype=f)
    half = S // 2
    invf = np.power(f(10000.0), (np.arange(half, dtype=f) / f(half)).astype(f)).astype(f)
    sinu = (pos[:, None] * invf[None, :]).astype(f)  # [n, half]
    c64 = np.cos(sinu.astype(np.float64)).T
    s64 = np.sin(sinu.astype(np.float64)).T
    cos_t = np.concatenate([c64, c64], axis=0).astype(f)  # [128, 512]
    sin_t = np.concatenate([-s64, s64], axis=0).astype(f)
    # q side absorbs the 1/MAX_SEQ of qk/MAX_SEQ
    cst = np.zeros((P, CST_W), dtype=f)
    cst[:, CST_COS : CST_COS + 512] = cos_t
    cst[:, CST_SIN : CST_SIN + 512] = sin_t
    cst[:, CST_BIAS : CST_BIAS + 2048] = biasT_dev.reshape(P, 2048)
    cst[:, CST_UB : CST_UB + 16] = uv_b_eff[:E].reshape(16, P).T
    cst[:, CST_BB] = uv_b_eff[2 * E :]
    cst[:, CST_GBQ] = gamma[0] / MAX_SEQ
    cst[:, CST_GBQ + 1] = beta[0] / MAX_SEQ
    cst[:, CST_GBK] = gamma[1]
    cst[:, CST_GBK + 1] = beta[1]
    cstr = np.zeros((1, E + HIDDEN + P), dtype=bf)
    cstr[0, :E] = uv_b_eff[E : 2 * E].astype(bf)
    cstr[0, E : E + HIDDEN] = o_b.astype(bf)
    cstr[0, E + HIDDEN :] = np.ones(P, dtype=bf)
    shared = {
        "w_u": w_u, "w_v": w_v, "w_b": w_b, "w_o": w_o,
        "cst": cst, "cstr": cstr,
    }
    in_maps = []
    for ci in range(N_CORES):
        m = dict(shared)
        m["x"] = np.ascontiguousarray(
            x[ci * B_LOC : (ci + 1) * B_LOC].reshape(NLOC, HIDDEN)
        )
        in_maps.append(m)
    return in_maps


def kernel(x, ln_w, ln_b, uv_w, uv_b, gamma, beta, o_w, o_b, w_rel):
    from concourse.bass_utils import run_bass_kernel_spmd

    if "nc" not in _CACHE:
        _CACHE["nc"] = _build()
    nc = _CACHE["nc"]
    in_maps = _host_prep(x, ln_w, ln_b, uv_w, uv_b, gamma, beta, o_w, o_b, w_rel)
    res = run_bass_kernel_spmd(nc, in_maps, core_ids=list(range(N_CORES)))
    out = np.concatenate(
        [res.results[ci]["out"].reshape(B_LOC, MAX_SEQ, HIDDEN) for ci in range(N_CORES)],
        axis=0,
    )
    return out.astype(np.float32)


# revision 15
# speedup vs baseline: 1.3421x; 1.0159x over previous
"""GateAttentionUnit Trainium2 kernel.

Full inputs in, full output out. Data-parallel over batch: 16 batches
split 2-per-core across 8 NeuronCores; each core runs an identical NEFF
(SPMD) on its own x slice with replicated weights.

Device math per core (2 pipelined batches of [512, 1024]):
  LN (ln affine folded into uv weights on host) -> xn (bf16),
  PE-transposed to xnT [d,n] -> uv projection via bf16 matmuls (base
  head / v natural / u transposed orientations) -> silu -> rope'd q,k
  (host sin/cos tables) -> qkT -> kernelT = relu(qkT + biasT)^2
  (toeplitz bias from host) -> kvT -> g = u * kv -> out proj + bias +
  shortcut.  One global 8-bank PSUM layout: acc0-3 output accumulators
  + w1-w4 working banks, so phases of consecutive batches overlap.
  Weights are host-re-laid-out for contiguous DMA; o_wT stays resident;
  weight DMAs issue on GpSimd, x/activations on Sync.
"""

import sys

if "/opt/trn_rl_repo" not in sys.path:
    sys.path.insert(0, "/opt/trn_rl_repo")

import numpy as np

MAX_SEQ = 512
HIDDEN = 1024
E = 2048
S = 128
EPS = 1e-5
N_CORES = 8
B = 16
B_LOC = B // N_CORES
P = 128
NLOC = B_LOC * MAX_SEQ

# packed f32 const blob layout (free-dim offsets)
CST_COS = 0
CST_SIN = 512
CST_BIAS = 1024          # [4, 512] flattened
CST_UB = 3072            # [16]
CST_BB = 3088            # [1]
CST_GBQ = 3089           # [2]
CST_GBK = 3091           # [2]
CST_W = 3093

_CACHE = {}


class _Ctx:
    pass


def _phase0_ln(c, b):
    """LayerNorm batch b and transpose into c.xnT [d-part, n-free] (bf16)."""
    import concourse.mybir as mybir

    nc = c.nc
    f32 = mybir.dt.float32
    bf16 = mybir.dt.bfloat16
    Alu = mybir.AluOpType
    Act = mybir.ActivationFunctionType
    r0 = b * MAX_SEQ
    c.xts = []
    for nt in range(4):
        xt = c.xres.tile([P, HIDDEN], f32, tag="xt", name=f"xt{b}_{nt}")
        c.xts.append(xt)
        nc.sync.dma_start(xt[:], c.x_ap[r0 + nt * P : r0 + (nt + 1) * P, :])
        s1 = c.stats.tile([P, 1], f32, tag="s1")
        nc.vector.tensor_reduce(s1[:], xt[:], mybir.AxisListType.X, Alu.add)
        sq = c.ln_pool.tile([P, HIDDEN], f32, tag="sq")
        s2 = c.stats.tile([P, 1], f32, tag="s2")
        nc.scalar.activation(sq[:], xt[:], Act.Square, accum_out=s2[:])
        mu = c.stats.tile([P, 1], f32, tag="mu")
        nc.vector.tensor_scalar_mul(mu[:], s1[:], 1.0 / HIDDEN)
        mu2 = c.stats.tile([P, 1], f32, tag="mu2")
        nc.vector.tensor_mul(mu2[:], mu[:], mu[:])
        # var = s2/H - mu^2
        var = c.stats.tile([P, 1], f32, tag="var")
        nc.vector.scalar_tensor_tensor(
            var[:], s2[:], 1.0 / HIDDEN, mu2[:], Alu.mult, Alu.subtract
        )
        sd = c.stats.tile([P, 1], f32, tag="sd")
        nc.scalar.activation(sd[:], var[:], Act.Sqrt, bias=c.eps_sb[:])
        rstd = c.stats.tile([P, 1], f32, tag="rstd")
        nc.vector.reciprocal(rstd[:], sd[:])
        nmu = c.stats.tile([P, 1], f32, tag="nmu")
        nc.vector.scalar_tensor_tensor(
            nmu[:], mu[:], -1.0, rstd[:], Alu.mult, Alu.mult
        )
        xn = c.ln_pool.tile([P, HIDDEN], bf16, tag="xn")
        nc.vector.tensor_scalar(xn[:], xt[:], rstd[:], nmu[:], Alu.mult, Alu.add)
        for dt_i in range(8):
            ps = c.ps.tile([P, P], bf16, tag="w4", name="tp")
            nc.tensor.transpose(ps[:], xn[:, dt_i * P : (dt_i + 1) * P], c.ident[:])
            nc.vector.tensor_copy(c.xnT[:, dt_i, nt * P : (nt + 1) * P], ps[:])


def _phase1_kernelT(c):
    """base head -> silu -> scale-offset -> rope -> qkT -> kernelT."""
    import concourse.mybir as mybir

    nc = c.nc
    f32 = mybir.dt.float32
    bf16 = mybir.dt.bfloat16
    Alu = mybir.AluOpType
    Act = mybir.ActivationFunctionType
    wb = c.wb_pool.tile([P, 8, S], bf16, tag="wb")
    nc.gpsimd.dma_start(wb[:], c.wb_ap)
    bps = c.ps.tile([P, MAX_SEQ], f32, tag="w4", name="bps")
    for k in range(8):
        nc.tensor.matmul(
            bps[:], wb[:, k, :], c.xnT[:, k, :], start=(k == 0), stop=(k == 7)
        )
    base_sb = c.qk1_pool.tile([P, MAX_SEQ], f32, tag="base")
    nc.scalar.activation(base_sb[:], bps[:], Act.Silu, bias=c.bb_sb)
    q_ro = c.qk1_pool.tile([P, MAX_SEQ], bf16, tag="q_ro")
    k_ro = c.qk1_pool.tile([P, MAX_SEQ], bf16, tag="k_ro")
    for gb0, gb1, ro in ((CST_GBQ, CST_GBQ + 1, q_ro), (CST_GBK, CST_GBK + 1, k_ro)):
        pre = c.qk_pool.tile([P, MAX_SEQ], f32, tag="pre")
        nc.vector.tensor_scalar(
            pre[:], base_sb[:],
            c.cst[:, gb0 : gb0 + 1], c.cst[:, gb1 : gb1 + 1],
            Alu.mult, Alu.add,
        )
        sw = c.qk_pool.tile([P, MAX_SEQ], f32, tag="sw")
        nc.sync.dma_start(sw[0:64, :], pre[64:128, :])
        nc.sync.dma_start(sw[64:128, :], pre[0:64, :])
        tmp = c.qk_pool.tile([P, MAX_SEQ], f32, tag="rtmp")
        nc.vector.tensor_mul(tmp[:], sw[:], c.sin_sb)
        ro32 = c.qk_pool.tile([P, MAX_SEQ], f32, tag="ro32")
        nc.vector.tensor_mul(ro32[:], pre[:], c.cos_sb)
        nc.vector.tensor_add(ro[:], ro32[:], tmp[:])
    kernelT = c.kern_pool.tile([P, 4, MAX_SEQ], bf16, tag="kernelT")
    for mt in range(4):
        qps = c.ps.tile([P, MAX_SEQ], f32, tag="w4", name="qps")
        nc.tensor.matmul(
            qps[:], k_ro[:, mt * P : (mt + 1) * P], q_ro[:], start=True, stop=True
        )
        t = c.qk_pool.tile([P, MAX_SEQ], f32, tag="kt_tmp")
        nc.vector.tensor_add(t[:], qps[:], c.bias_sb[:, mt, :])
        nc.vector.scalar_tensor_tensor(
            kernelT[:, mt, :], t[:], 0.0, t[:], Alu.max, Alu.mult
        )
    c.kernelT = kernelT


def _phase2_gated(c):
    """v chunks, u tiles, kvT, g = u * kv into c.gT."""
    import concourse.mybir as mybir

    nc = c.nc
    f32 = mybir.dt.float32
    bf16 = mybir.dt.bfloat16
    Act = mybir.ActivationFunctionType
    for ec in range(4):
        wv = c.wv_pool.tile([P, 8, 512], bf16, tag="wv")
        nc.gpsimd.dma_start(wv[:], c.wv_ap[ec])
        vch = c.v_pool.tile([P, 4, 512], bf16, tag="vch")
        for nt in range(4):
            vps = c.ps.tile([P, 512], f32, tag="w1", name="vps")
            for k in range(8):
                nc.tensor.matmul(
                    vps[:], c.xnT[:, k, nt * P : (nt + 1) * P], wv[:, k, :],
                    start=(k == 0), stop=False,
                )
            nc.tensor.matmul(
                vps[:], c.ones_sb, c.vb_sb[:, ec * 512 : (ec + 1) * 512],
                start=False, stop=True,
            )
            nc.scalar.activation(vch[:, nt, :], vps[:], Act.Silu)
        for eip in range(2):  # ei pairs within chunk
            wu = c.wu_pool.tile([P, 2, 8, P], bf16, tag="wu")
            nc.gpsimd.dma_start(wu[:], c.wu_ap[ec * 2 + eip])
            if c.load_wo:
                woi = ec * 2 + eip
                nc.gpsimd.dma_start(
                    c.wo_sb[:, 2 * woi : 2 * woi + 2, :],
                    c.wo_ap[2 * woi : 2 * woi + 2].rearrange("e p d -> p e d"),
                )
            for sub in range(2):
                ei = ec * 4 + eip * 2 + sub
                et = eip * 2 + sub
                ups = c.ps.tile([P, MAX_SEQ], f32, tag="w2", name="ups")
                for k in range(8):
                    nc.tensor.matmul(
                        ups[:], wu[:, sub, k, :], c.xnT[:, k, :],
                        start=(k == 0), stop=(k == 7),
                    )
                ut = c.u_pool.tile([P, MAX_SEQ], f32, tag="ut")
                nc.scalar.activation(
                    ut[:], ups[:], Act.Silu, bias=c.ub_sb[:, ei : ei + 1]
                )
                kps = c.ps.tile([P, MAX_SEQ], f32, tag="w3", name="kps")
                for mt in range(4):
                    nc.tensor.matmul(
                        kps[:], vch[:, mt, et * P : (et + 1) * P],
                        c.kernelT[:, mt, :],
                        start=(mt == 0), stop=(mt == 3),
                    )
                nc.vector.tensor_mul(c.gT[:, ei, :], ut[:], kps[:])


def _phase3_out(c, b):
    """out = gT.T @ o_wT + o_b + shortcut, two half-d sweeps of 4 accumulators."""
    import concourse.mybir as mybir

    nc = c.nc
    f32 = mybir.dt.float32
    r0 = b * MAX_SEQ
    for dc in range(2):
        accs = [
            c.ps.tile([P, 512], f32, tag=f"acc{i}", name=f"acc{i}") for i in range(4)
        ]
        for et in range(16):
            for nt in range(4):
                nc.tensor.matmul(
                    accs[nt][:], c.gT[:, et, nt * P : (nt + 1) * P],
                    c.wo_sb[:, et, dc * 512 : (dc + 1) * 512],
                    start=(et == 0), stop=False,
                )
        for nt in range(4):
            nc.tensor.matmul(
                accs[nt][:], c.ones_sb, c.ob_sb[:, dc * 512 : (dc + 1) * 512],
                start=False, stop=True,
            )
            osb = c.out_pool.tile([P, 512], f32, tag="osb")
            nc.vector.tensor_add(
                osb[:], accs[nt][:], c.xts[nt][:, dc * 512 : (dc + 1) * 512]
            )
            nc.gpsimd.dma_start(
                c.out_ap[r0 + nt * P : r0 + (nt + 1) * P, dc * 512 : (dc + 1) * 512],
                osb[:],
            )


def _build():
    import concourse.mybir as mybir
    import concourse.tile as tile
    from concourse import bacc
    from concourse.masks import make_identity

    f32 = mybir.dt.float32
    bf16 = mybir.dt.bfloat16

    nc = bacc.Bacc("TRN2", target_bir_lowering=False, debug=False,
                   num_devices=N_CORES)

    x_d = nc.dram_tensor("x", [NLOC, HIDDEN], f32, kind="ExternalInput")
    wu_d = nc.dram_tensor("w_u", [16, P, 8, P], bf16, kind="ExternalInput")
    wv_d = nc.dram_tensor("w_v", [4, P, 8, 512], bf16, kind="ExternalInput")
    wb_d = nc.dram_tensor("w_b", [P, 8, S], bf16, kind="ExternalInput")
    wo_d = nc.dram_tensor("w_o", [16, P, HIDDEN], bf16, kind="ExternalInput")
    cst_d = nc.dram_tensor("cst", [P, CST_W], f32, kind="ExternalInput")
    cstr_d = nc.dram_tensor("cstr", [1, E + HIDDEN + P], bf16, kind="ExternalInput")
    out_d = nc.dram_tensor("out", [NLOC, HIDDEN], f32, kind="ExternalOutput")

    c = _Ctx()
    c.nc = nc
    c.x_ap = x_d.ap()
    c.out_ap = out_d.ap()
    c.wb_ap = wb_d.ap()
    c.wu_ap = [
        wu_d.ap()[2 * i : 2 * i + 2].rearrange("e p do f -> p e do f") for i in range(8)
    ]
    c.wv_ap = [wv_d.ap()[i] for i in range(4)]
    c.wo_ap = wo_d.ap()

    with tile.TileContext(nc) as tc:
        c.tc = tc
        with (
            tc.tile_pool(name="const", bufs=1) as const,
            tc.tile_pool(name="xres", bufs=5) as xres,
            tc.tile_pool(name="ln", bufs=2) as ln_pool,
            tc.tile_pool(name="stats", bufs=4) as stats,
            tc.tile_pool(name="xnT", bufs=2) as xnT_pool,
            tc.tile_pool(name="wb", bufs=2) as wb_pool,
            tc.tile_pool(name="qk1", bufs=1) as qk1_pool,
            tc.tile_pool(name="qk2", bufs=1) as qk_pool,
            tc.tile_pool(name="kern", bufs=2) as kern_pool,
            tc.tile_pool(name="wv", bufs=2) as wv_pool,
            tc.tile_pool(name="vch", bufs=2) as v_pool,
            tc.tile_pool(name="wu", bufs=3) as wu_pool,
            tc.tile_pool(name="ut", bufs=3) as u_pool,
            tc.tile_pool(name="gT", bufs=2) as gT_pool,
            tc.tile_pool(name="outs", bufs=3) as out_pool,
            tc.tile_pool(name="ps", bufs=1, space="PSUM") as ps,
        ):
            c.xres = xres
            c.ln_pool = ln_pool
            c.stats = stats
            c.wb_pool = wb_pool
            c.qk1_pool = qk1_pool
            c.qk_pool = qk_pool
            c.kern_pool = kern_pool
            c.wv_pool = wv_pool
            c.v_pool = v_pool
            c.wu_pool = wu_pool
            c.u_pool = u_pool
            c.out_pool = out_pool
            c.ps = ps

            c.ident = const.tile([P, P], bf16)
            make_identity(nc, c.ident[:])
            c.eps_sb = const.tile([P, 1], f32)
            nc.vector.memset(c.eps_sb[:], EPS)

            cst = const.tile([P, CST_W], f32)
            nc.sync.dma_start(cst[:], cst_d.ap())
            c.cst = cst
            c.cos_sb = cst[:, CST_COS : CST_COS + 512]
            c.sin_sb = cst[:, CST_SIN : CST_SIN + 512]
            c.bias_sb = cst[:, CST_BIAS : CST_BIAS + 2048].rearrange(
                "p (mt n) -> p mt n", mt=4
            )
            c.ub_sb = cst[:, CST_UB : CST_UB + 16]
            c.bb_sb = cst[:, CST_BB : CST_BB + 1]

            cstr = const.tile([1, E + HIDDEN + P], bf16)
            nc.sync.dma_start(cstr[:], cstr_d.ap())
            c.vb_sb = cstr[:, 0:E]
            c.ob_sb = cstr[:, E : E + HIDDEN]
            c.ones_sb = cstr[:, E + HIDDEN : E + HIDDEN + P]

            c.wo_sb = const.tile([P, 16, HIDDEN], bf16)

            for b in range(B_LOC):
                c.load_wo = b == 0
                c.xnT = xnT_pool.tile([P, 8, MAX_SEQ], bf16, tag="xnT", name="xnT")
                _phase0_ln(c, b)
                _phase1_kernelT(c)
                c.gT = gT_pool.tile([P, 16, MAX_SEQ], bf16, tag="gT", name="gT")
                _phase2_gated(c)
                _phase3_out(c, b)

    nc.compile()
    return nc


def _host_prep(x, ln_w, ln_b, uv_w, uv_b, gamma, beta, o_w, o_b, w_rel):
    import ml_dtypes

    f = np.float32
    bf = ml_dtypes.bfloat16
    x = np.ascontiguousarray(x, dtype=f)
    # fold LN affine into the uv projection (exact algebra)
    uv_wT = (uv_w * ln_w[None, :]).T.astype(bf)  # [1024, 4224]
    uv_b_eff = (
        uv_b.astype(np.float64) + uv_w.astype(np.float64) @ ln_b.astype(np.float64)
    ).astype(f)
    # weight re-layouts for contiguous DMA
    w_u = np.ascontiguousarray(
        uv_wT[:, :E].reshape(8, P, 16, P).transpose(2, 1, 0, 3)
    )  # [ei, di, do, f]
    w_v = np.ascontiguousarray(
        uv_wT[:, E : 2 * E].reshape(8, P, 4, 512).transpose(2, 1, 0, 3)
    )  # [ec, di, do, f]
    w_b = np.ascontiguousarray(
        uv_wT[:, 2 * E :].reshape(8, P, S).transpose(1, 0, 2)
    )  # [di, do, f]
    w_o = np.ascontiguousarray(o_w.T.astype(bf).reshape(16, P, HIDDEN))  # [et, p, d]
    # toeplitz bias, transposed: biasT[m, n] = bias[n, m] = w_rel[511 + m - n]
    idx = (MAX_SEQ - 1) + np.arange(MAX_SEQ)[:, None] - np.arange(MAX_SEQ)[None, :]
    biasT = w_rel[idx].astype(f)
    biasT_dev = biasT.reshape(4, P, MAX_SEQ).transpose(1, 0, 2)  # [p, mt, n]
    # rope tables (fp32 sinusoid to match reference rounding, accurate sin/cos)
    pos = np.arange(MAX_SEQ, dtype=f)
    half = S // 2
    invf = np.power(f(10000.0), (np.arange(half, dtype=f) / f(half)).astype(f)).astype(f)
    sinu = (pos[:, None] * invf[None, :]).astype(f)  # [n, half]
    c64 = np.cos(sinu.astype(np.float64)).T
    s64 = np.sin(sinu.astype(np.float64)).T
    cos_t = np.concatenate([c64, c64], axis=0).astype(f)  # [128, 512]
    sin_t = np.concatenate([-s64, s64], axis=0).astype(f)
    # q side absorbs the 1/MAX_SEQ of qk/MAX_SEQ
    cst = np.zeros((P, CST_W), dtype=f)
    cst[:, CST_COS : CST_COS + 512] = cos_t
    cst[:, CST_SIN : CST_SIN + 512] = sin_t
    cst[:, CST_BIAS : CST_BIAS + 2048] = biasT_dev.reshape(P, 2048)
    cst[:, CST_UB : CST_UB + 16] = uv_b_eff[:E].reshape(16, P).T
    cst[:, CST_BB] = uv_b_eff[2 * E :]
    cst[:, CST_GBQ] = gamma[0] / MAX_SEQ
    cst[:, CST_GBQ + 1] = beta[0] / MAX_SEQ
    cst[:, CST_GBK] = gamma[1]
    cst[:, CST_GBK + 1] = beta[1]
    cstr = np.zeros((1, E + HIDDEN + P), dtype=bf)
    cstr[0, :E] = uv_b_eff[E : 2 * E].astype(bf)
    cstr[0, E : E + HIDDEN] = o_b.astype(bf)
    cstr[0, E + HIDDEN :] = np.ones(P, dtype=bf)
    shared = {
        "w_u": w_u, "w_v": w_v, "w_b": w_b, "w_o": w_o,
        "cst": cst, "cstr": cstr,
    }
    in_maps = []
    for ci in range(N_CORES):
        m = dict(shared)
        m["x"] = np.ascontiguousarray(
            x[ci * B_LOC : (ci + 1) * B_LOC].reshape(NLOC, HIDDEN)
        )
        in_maps.append(m)
    return in_maps


def kernel(x, ln_w, ln_b, uv_w, uv_b, gamma, beta, o_w, o_b, w_rel):
    from concourse.bass_utils import run_bass_kernel_spmd

    if "nc" not in _CACHE:
        _CACHE["nc"] = _build()
    nc = _CACHE["nc"]
    in_maps = _host_prep(x, ln_w, ln_b, uv_w, uv_b, gamma, beta, o_w, o_b, w_rel)
    res = run_bass_kernel_spmd(nc, in_maps, core_ids=list(range(N_CORES)))
    out = np.concatenate(
        [res.results[ci]["out"].reshape(B_LOC, MAX_SEQ, HIDDEN) for ci in range(N_CORES)],
        axis=0,
    )
    return out.astype(np.float32)
